# revision 5
# baseline (speedup 1.0000x reference)
"""Trainium2 Bass kernel for nn_DS_Attention_7636451852327.

Data-parallel over batch: 32 batches -> 8 NeuronCores, 4 batches (2048 tokens)
per core, 16 token-tiles of 128.

Host-side prep: q/v shipped pre-transposed ([512, T] fp16) so the QKV matmul
lhsT tiles are direct DMA loads (no on-device cast / PE transpose / PSUM
copy).  lin_w rows are permuted so the attention output is written in
(h, d, q) order (keeps the normalization multiply in DVE 2x mode), and the
output bias is folded into the final matmul via an appended ones-row.

Device-side inner attention (per 128-token tile, tokens on partitions):
  p1 = qa*ka broadcast products      (merged across heads, 4-dim APs)
  d-sum via [.,2]-lane adds, e = exp(lane0)*exp(lane1) (exps on ACT)
  rowsum tree / recip / custom-weighting chains (on unnormalized e + u)
  p2 = e*va broadcast products, k-sum tree, a0*r + rank-1 du correction
All elementwise work is fp16 (DVE 2x mode); a configurable head-split
offloads part of the bulk work to the Pool (GPSIMD) engine so DVE and Pool
run concurrently.
"""
import os as _os
import numpy as np
from contextlib import ExitStack

import concourse.bass as bass
import concourse.mybir as mybir
import concourse.tile as tile
from concourse import bacc
from concourse.bass_utils import run_bass_kernel_spmd
from concourse.masks import make_identity

hp = mybir.dt.float16
f32 = mybir.dt.float32
AL = mybir.AluOpType
AX = mybir.AxisListType
AF = mybir.ActivationFunctionType

P = 128
H = 8
QJ = KJ = 24
D = 6
NQK = QJ * D              # 144
NVA = H * NQK             # 1152
NP = QJ * KJ              # 576 (q,k) pairs per head
DH = 1176                 # 147*8
D_MODEL = 512
W_TOT = 3 * NVA + H * 3   # 3480
B = 32
N = 512
N_CORES = 8
TT = (B // N_CORES) * N // P   # 16 token tiles per core

# columns touched by the custom-weighting chains (row-0 dsts / col-0 dsts)
CH_LEVELS = (((6, 7), (3, 4)), ((9, 10), (6, 7)),
             ((12, 15), (9, 10)), ((15, 18), (12, 15)))

QKV_CHUNKS = [
    (0, 512, 0), (512, 1024, 0), (1024, 1152, 0),
    (1152, 1664, 1), (1664, 2176, 1), (2176, 2304, 1),
    (2304, 2816, 2), (2816, 3328, 2), (3328, 3480, 2),
]

# DVE/Pool head splits: heads [0, X_DVE) run on DVE, the rest on Pool.
def _cfg(name, default):
    v = _os.environ.get(name)
    return int(v) if v else default

P1_DVE = _cfg("P1_DVE", 8)
S2A_DVE = _cfg("S2A_DVE", 7)
EM_DVE = _cfg("EM_DVE", 6)
RS_DVE = _cfg("RS_DVE", 6)
P2_DVE = _cfg("P2_DVE", 8)
KJ_DVE = _cfg("KJ_DVE", 4)
CH_POOL = _cfg("CH_POOL", 0)
NG = _cfg("NG", 2)  # head-group count for p1/s2a interleaving with ACT exps


def _split_groups(ndve, ngroups):
    """head ranges [(lo,hi,engine_is_pool)] covering 0..H."""
    out = []
    if ndve > 0:
        step = max(1, (ndve + ngroups - 1) // ngroups)
        lo = 0
        while lo < ndve:
            hi = min(ndve, lo + step)
            out.append((lo, hi, False))
            lo = hi
    if ndve < H:
        out.append((ndve, H, True))
    return out


def build_program(tt=TT, inner_repeat=1):
    nc = bacc.Bacc("TRN2", target_bir_lowering=False, debug=False)
    T = tt * P
    qT_dram = nc.dram_tensor("qT", [D_MODEL, T], hp, kind="ExternalInput").ap()
    vT_dram = nc.dram_tensor("vT", [D_MODEL, T], hp, kind="ExternalInput").ap()
    wcat_dram = nc.dram_tensor("w_cat", [D_MODEL, W_TOT], hp, kind="ExternalInput").ap()
    lw_dram = nc.dram_tensor("lin_w", [1184, D_MODEL], hp, kind="ExternalInput").ap()
    out_dram = nc.dram_tensor("out", [T, D_MODEL], f32, kind="ExternalOutput").ap()

    with tile.TileContext(nc) as tc, ExitStack() as ctx:
        const = ctx.enter_context(tc.tile_pool(name="const", bufs=1))
        wpool = ctx.enter_context(tc.tile_pool(name="wpool", bufs=1))
        io = ctx.enter_context(tc.tile_pool(name="io", bufs=2))
        qkv = ctx.enter_context(tc.tile_pool(name="qkv", bufs=1))
        vpt = ctx.enter_context(tc.tile_pool(name="vpt", bufs=2))
        big = ctx.enter_context(tc.tile_pool(name="big", bufs=1))
        small = ctx.enter_context(tc.tile_pool(name="small", bufs=1))
        vt = ctx.enter_context(tc.tile_pool(name="vt", bufs=1))
        outp = ctx.enter_context(tc.tile_pool(name="outp", bufs=2))
        ps_t = ctx.enter_context(tc.tile_pool(name="ps_t", bufs=2, space="PSUM"))
        ps_mm = ctx.enter_context(tc.tile_pool(name="ps_mm", bufs=2, space="PSUM"))
        ps_out = ctx.enter_context(tc.tile_pool(name="ps_out", bufs=2, space="PSUM"))

        ident = const.tile([P, P], hp, tag="ident")
        make_identity(nc, ident[:])
        wcat = []
        for k in range(4):
            wk = wpool.tile([P, W_TOT], hp, tag=f"wcat{k}")
            nc.sync.dma_start(wk[:], wcat_dram[k * P:(k + 1) * P, :])
            wcat.append(wk)
        lw = []
        for k in range(10):
            rows = min(P, DH - k * P)
            if k == 9:
                rows += 1  # bias row
            lwk = wpool.tile([P, D_MODEL], hp, tag=f"lw{k}")
            nc.sync.dma_start(lwk[:rows, :], lw_dram[k * P:k * P + rows, :])
            lw.append((lwk, rows))

        for it in range(tt):
          for _rep in range(inner_repeat):
            # ---- input tiles: direct transposed fp16 loads ----
            xq, xv = [], []
            for src, dst, nm in ((qT_dram, xq, "q"), (vT_dram, xv, "v")):
                for k in range(4):
                    xk = io.tile([P, P], hp, tag=f"x{nm}{k}")
                    nc.sync.dma_start(xk[:], src[k * P:(k + 1) * P, it * P:(it + 1) * P])
                    dst.append(xk)

            # ---- QKV projection (token-major) ----
            qa_all = qkv.tile([P, NVA], hp, tag="qa_all")
            ka_all = qkv.tile([P, NVA], hp, tag="ka_all")
            va_all = qkv.tile([P, NVA], hp, tag="va_all")
            vptok = vpt.tile([P, DH + 1], hp, tag="vptok")
            nc.gpsimd.memset(vptok[:, DH:DH + 1], 1.0)  # ones col -> bias row of v'^T
            for (c0, c1, kind) in QKV_CHUNKS:
                w_n = c1 - c0
                pmm = ps_mm.tile([P, 512], f32, tag="pmm")
                lhs_tiles = xv if kind == 2 else xq
                for k in range(4):
                    nc.tensor.matmul(pmm[:, :w_n], lhs_tiles[k][:], wcat[k][:, c0:c1],
                                     start=(k == 0), stop=(k == 3))
                if kind == 0:
                    nc.scalar.copy(qa_all[:, c0:c1], pmm[:, :w_n])
                elif kind == 1:
                    nc.scalar.copy(ka_all[:, c0 - NVA:c1 - NVA], pmm[:, :w_n])
                else:
                    v0, v1 = c0 - 2 * NVA, c1 - 2 * NVA
                    if v1 <= NVA:
                        nc.scalar.copy(va_all[:, v0:v1], pmm[:, :w_n])
                    else:
                        nc.scalar.copy(va_all[:, v0:NVA], pmm[:, :NVA - v0])
                        vp = pmm[:, NVA - v0:w_n].rearrange("p (h c) -> p h c", h=H)
                        vp_dst = vptok[:, :DH].rearrange("p (h c) -> p h c", h=H)[:, :, :3]
                        nc.scalar.copy(vp_dst, vp)

            # ---- inner attention (merged heads, DVE/Pool split) ----
            p12 = big.tile([P, H * QJ * KJ * D], hp, tag="p12")   # p1, later p2
            s2a = big.tile([P, H * NP * 2], hp, tag="s2a")        # later t6 tree
            ea = big.tile([P, H * NP], hp, tag="ea")
            eb = big.tile([P, H * NP], hp, tag="eb")
            e_all = big.tile([P, H * NP], hp, tag="e_all")
            t12 = big.tile([P, H * NQK * 12], hp, tag="t12")
            r12 = small.tile([P, H * QJ * 12], hp, tag="r12")     # later t2 tree
            r6 = small.tile([P, H * QJ * 6], hp, tag="r6")
            r2t = small.tile([P, H * QJ * 2], hp, tag="r2t")
            s_all = small.tile([P, H * QJ], f32, tag="s_all")
            r16 = small.tile([P, H * QJ], hp, tag="r16")
            u_all = small.tile([P, H * QJ], hp, tag="u_all")
            u2 = small.tile([P, H * QJ], hp, tag="u2")
            du = small.tile([P, H * QJ], hp, tag="du")
            a0 = small.tile([P, H * D * QJ], hp, tag="a0")
            ct = small.tile([P, H * D * D], hp, tag="ct")

            qa_v = qa_all[:].rearrange("p (h q d) -> p h q d", h=H, q=QJ)
            ka_v = ka_all[:].rearrange("p (h k d) -> p h k d", h=H, k=KJ)
            va_v = va_all[:].rearrange("p (h d k) -> p h d k", h=H, d=D)
            p1_v = p12[:].rearrange("p (h q k d) -> p h q k d", h=H, q=QJ, k=KJ)
            p1_f = p12[:].rearrange("p (h pr d) -> p h pr d", h=H, pr=NP)
            s2_v = s2a[:].rearrange("p (h pr e) -> p h pr e", h=H, pr=NP)
            e_v = e_all[:].rearrange("p (h q k) -> p h q k", h=H, q=QJ)

            # p1 products + d-sum pairs, grouped for exp interleave
            for (lo, hi, on_pool) in _split_groups(P1_DVE, NG):
                eng = nc.gpsimd if on_pool else nc.vector
                nh = hi - lo
                qa_b = qa_v[:, lo:hi].unsqueeze(3).broadcast_to([P, nh, QJ, KJ, D])
                ka_b = ka_v[:, lo:hi].unsqueeze(2).broadcast_to([P, nh, QJ, KJ, D])
                eng.tensor_tensor(p1_v[:, lo:hi], qa_b, ka_b, AL.mult)
                for (slo, shi, s_pool) in _split_groups(S2A_DVE, 1):
                    glo, ghi = max(lo, slo), min(hi, shi)
                    if glo >= ghi:
                        continue
                    seng = nc.gpsimd if s_pool else nc.vector
                    seng.tensor_tensor(s2_v[:, glo:ghi], p1_f[:, glo:ghi, :, 0:2],
                                       p1_f[:, glo:ghi, :, 2:4], AL.add)
                    seng.tensor_tensor(s2_v[:, glo:ghi], s2_v[:, glo:ghi],
                                       p1_f[:, glo:ghi, :, 4:6], AL.add)
                # exps for this group's s2a (ACT)
                nc.scalar.activation(ea[:, lo * NP:hi * NP], s2_v[:, lo:hi, :, 0], AF.Exp)
                nc.scalar.activation(eb[:, lo * NP:hi * NP], s2_v[:, lo:hi, :, 1], AF.Exp)
            # e = ea * eb
            for (lo, hi, on_pool) in _split_groups(EM_DVE, 1):
                eng = nc.gpsimd if on_pool else nc.vector
                eng.tensor_tensor(e_all[:, lo * NP:hi * NP], ea[:, lo * NP:hi * NP],
                                  eb[:, lo * NP:hi * NP], AL.mult)

            # rowsum over k: 24 -> 12 -> 6 -> 2 -> 1 (pre-chain e)
            HQ = H * QJ
            ef = e_all[:].rearrange("p (hq k) -> p hq k", hq=HQ)
            r12v = r12[:].rearrange("p (f k) -> p f k", f=HQ)
            r6v = r6[:].rearrange("p (f k) -> p f k", f=HQ)
            r2v = r2t[:].rearrange("p (f k) -> p f k", f=HQ)
            for (lo, hi, on_pool) in _split_groups(RS_DVE, 1):
                eng = nc.gpsimd if on_pool else nc.vector
                fl, fh = lo * QJ, hi * QJ
                eng.tensor_tensor(r12v[:, fl:fh], ef[:, fl:fh, 0:12], ef[:, fl:fh, 12:24], AL.add)
                eng.tensor_tensor(r6v[:, fl:fh], r12v[:, fl:fh, 0:6], r12v[:, fl:fh, 6:12], AL.add)
                eng.tensor_tensor(r2v[:, fl:fh], r6v[:, fl:fh, 0:2], r6v[:, fl:fh, 2:4], AL.add)
                eng.tensor_tensor(r2v[:, fl:fh], r2v[:, fl:fh], r6v[:, fl:fh, 4:6], AL.add)
                with nc.allow_low_precision(reason="fp16 attention"):
                    eng.tensor_tensor(s_all[:, fl:fh], r2v[:, fl:fh, 0], r2v[:, fl:fh, 1], AL.add)
            with nc.allow_low_precision(reason="fp16 attention"):
                nc.vector.reciprocal(r16[:], s_all[:])

            # u = p[:, 0] = e[:, :, q, 0] * r ; u2 = chained copy ; du
            rv = r16[:].rearrange("p (h q) -> p h q", h=H)
            uv = u2[:].rearrange("p (h q) -> p h q", h=H)
            nc.vector.tensor_tensor(
                u_all[:].rearrange("p (h q) -> p h q", h=H), e_v[:, :, :, 0], rv, AL.mult)
            nc.scalar.copy(u2[:], u_all[:])

            # custom-weighting chains: e row q=0 (pre-normalization) and u
            ch_eng = nc.gpsimd if CH_POOL else nc.vector
            tmp8 = small.tile([P, H * 3], hp, tag="tmp8")
            t3 = tmp8[:].rearrange("p (h c) -> p h c", h=H)
            for view in (e_v[:, :, 0, :], uv):
                for dsl, ssl in CH_LEVELS:
                    nd = dsl[1] - dsl[0]
                    dst = view[:, :, dsl[0]:dsl[1]]
                    src = view[:, :, ssl[0]:ssl[1]]
                    if ssl[1] - ssl[0] < nd:
                        src = src.broadcast_to([P, H, nd])
                    ch_eng.tensor_tensor(t3[:, :, :nd], dst, src, AL.add)
                    ch_eng.tensor_scalar_mul(dst, t3[:, :, :nd], 0.5)
            nc.vector.tensor_tensor(du[:], u2[:], u_all[:], AL.subtract)

            # p2 = e * va  (h, d, q, k) then k-sum tree 24->12->6->2->1
            p2_v = p12[:].rearrange("p (h d q k) -> p h d q k", h=H, d=D, q=QJ)
            p2_f = p12[:].rearrange("p (h f k) -> p h f k", h=H, f=NQK)
            t12v = t12[:].rearrange("p (h f k) -> p h f k", h=H, f=NQK)
            t6v = s2a[:, :H * NQK * 6].rearrange("p (h f k) -> p h f k", h=H, f=NQK)
            t2v = r12[:, :H * NQK * 2].rearrange("p (h f k) -> p h f k", h=H, f=NQK)
            a0v = a0[:].rearrange("p (h d q) -> p h d q", h=H, d=D)
            for (lo, hi, on_pool) in _split_groups(P2_DVE, 1):
                eng = nc.gpsimd if on_pool else nc.vector
                nh = hi - lo
                e_b = e_v[:, lo:hi].unsqueeze(2).broadcast_to([P, nh, D, QJ, KJ])
                va_b = va_v[:, lo:hi].unsqueeze(3).broadcast_to([P, nh, D, QJ, KJ])
                eng.tensor_tensor(p2_v[:, lo:hi], e_b, va_b, AL.mult)
            for (lo, hi, on_pool) in _split_groups(KJ_DVE, 1):
                eng = nc.gpsimd if on_pool else nc.vector
                eng.tensor_tensor(t12v[:, lo:hi], p2_f[:, lo:hi, :, 0:12],
                                  p2_f[:, lo:hi, :, 12:24], AL.add)
                eng.tensor_tensor(t6v[:, lo:hi], t12v[:, lo:hi, :, 0:6],
                                  t12v[:, lo:hi, :, 6:12], AL.add)
                eng.tensor_tensor(t2v[:, lo:hi], t6v[:, lo:hi, :, 0:2],
                                  t6v[:, lo:hi, :, 2:4], AL.add)
                eng.tensor_tensor(t2v[:, lo:hi], t2v[:, lo:hi], t6v[:, lo:hi, :, 4:6], AL.add)
                eng.tensor_tensor(a0v[:, lo:hi].rearrange("p h d q -> p h (d q)"),
                                  t2v[:, lo:hi, :, 0], t2v[:, lo:hi, :, 1], AL.add)

            # att = a0 * r (+ du x va0 on chain rows); vptok att layout (h, d, q)
            att = vptok[:, :DH].rearrange("p (h c) -> p h c", h=H)[:, :, 3:]
            attv = att.rearrange("p h (d q) -> p h d q", d=D)
            r_b = rv.unsqueeze(2).broadcast_to([P, H, D, QJ])
            nc.vector.tensor_tensor(attv, a0v, r_b, AL.mult)
            du_v = du[:].rearrange("p (h q) -> p h q", h=H)
            va0 = va_v[:, :, :, 0]
            ctv = ct[:].rearrange("p (h d q) -> p h d q", h=H, d=D)
            # q in 12..17 (packed run) and q in {6, 9} (stride 3)
            for qsl, n_q in (((12, 18, 1), 6), ((6, 10, 3), 2)):
                du_b = du_v[:, :, qsl[0]:qsl[1]:qsl[2]].unsqueeze(2).broadcast_to([P, H, D, n_q])
                va0_b = va0.unsqueeze(3).broadcast_to([P, H, D, n_q])
                nc.vector.tensor_tensor(ctv[:, :, :, :n_q], du_b, va0_b, AL.mult)
                nc.vector.tensor_tensor(attv[:, :, :, qsl[0]:qsl[1]:qsl[2]],
                                        attv[:, :, :, qsl[0]:qsl[1]:qsl[2]],
                                        ctv[:, :, :, :n_q], AL.add)

            # ---- output transposes + final matmul (bias folded via ones row) ----
            vT = []
            for k in range(10):
                cols = min(P, DH + 1 - k * P)
                pst2 = ps_t.tile([P, P], hp, tag="pst2")
                nc.tensor.transpose(pst2[:cols, :], vptok[:, k * P:k * P + cols], ident[:])
                vk = vt.tile([P, P], hp, tag=f"vT{k}")
                nc.scalar.copy(vk[:cols, :], pst2[:cols, :])
                vT.append((vk, cols))
            pout = ps_out.tile([P, D_MODEL], f32, tag="pout")
            for k in range(10):
                vk, rows = vT[k]
                lwk, rows2 = lw[k]
                nc.tensor.matmul(pout[:], vk[:rows, :], lwk[:rows, :],
                                 start=(k == 0), stop=(k == 9))
            out_sb = outp.tile([P, D_MODEL], f32, tag="out_sb")
            nc.scalar.copy(out_sb[:], pout[:])
            nc.sync.dma_start(out_dram[it * P:(it + 1) * P, :], out_sb[:])

    nc.compile()
    return nc


def prep_weights(qk_w, v_w, lin_w, lin_b):
    scale = np.float32(1.0 / np.sqrt(6.0))
    wq = np.asarray(qk_w[:, :DH], dtype=np.float32).reshape(D_MODEL, H, 147)
    wk = np.asarray(qk_w[:, DH:], dtype=np.float32).reshape(D_MODEL, H, 147)
    wv = np.asarray(v_w, dtype=np.float32).reshape(D_MODEL, H, 147)
    wq_p = (wq[:, :, 3:] * scale).reshape(D_MODEL, H * NQK)
    wk_p = wk[:, :, 3:].reshape(D_MODEL, H * NQK)
    wv_att = wv[:, :, 3:].reshape(D_MODEL, H, KJ, D).transpose(0, 1, 3, 2).reshape(D_MODEL, H * NQK)
    wv_pass = wv[:, :, :3].reshape(D_MODEL, H * 3)
    w_cat = np.ascontiguousarray(
        np.concatenate([wq_p, wk_p, wv_att, wv_pass], axis=1)).astype(np.float16)
    # lin_w rows permuted to the (h, [pass3, d*24+q]) vptok layout + bias row
    lwr = np.asarray(lin_w, dtype=np.float32).reshape(H, 147, D_MODEL)
    att = lwr[:, 3:, :].reshape(H, QJ, D, D_MODEL).transpose(0, 2, 1, 3).reshape(H, NQK, D_MODEL)
    lw_p = np.concatenate([lwr[:, :3, :], att], axis=1).reshape(DH, D_MODEL)
    lw_aug = np.zeros((1184, D_MODEL), dtype=np.float32)
    lw_aug[:DH] = lw_p
    lw_aug[DH] = np.asarray(lin_b, dtype=np.float32)
    return w_cat, np.ascontiguousarray(lw_aug).astype(np.float16)


def make_in_maps(query, value, qk_w, v_w, lin_w, lin_b):
    w_cat, lw_aug = prep_weights(qk_w, v_w, lin_w, lin_b)
    q = np.asarray(query, dtype=np.float32)
    v = np.asarray(value, dtype=np.float32)
    bpc = B // N_CORES
    in_maps = []
    for c in range(N_CORES):
        qc = q[c * bpc:(c + 1) * bpc].reshape(-1, D_MODEL).T
        vc = v[c * bpc:(c + 1) * bpc].reshape(-1, D_MODEL).T
        in_maps.append({
            "qT": np.ascontiguousarray(qc).astype(np.float16),
            "vT": np.ascontiguousarray(vc).astype(np.float16),
            "w_cat": w_cat,
            "lin_w": lw_aug,
        })
    return in_maps


_CACHED_NC = None


def _get_nc():
    global _CACHED_NC
    if _CACHED_NC is None:
        _CACHED_NC = build_program(TT)
    return _CACHED_NC


def kernel(query, key, value, qk_w, v_w, lin_w, lin_b, _want_results=False, **_ignored):
    """Full-input kernel: shards batch over 8 cores, returns full output."""
    in_maps = make_in_maps(query, value, qk_w, v_w, lin_w, lin_b)
    nc = _get_nc()
    bpc = B // N_CORES
    res = run_bass_kernel_spmd(nc, in_maps, core_ids=list(range(N_CORES)))
    out = np.empty((B, N, D_MODEL), dtype=np.float32)
    for c in range(N_CORES):
        out[c * bpc:(c + 1) * bpc] = res.results[c]["out"].reshape(bpc, N, D_MODEL)
    if _want_results:
        return out, res
    return out


# revision 11
# speedup vs baseline: 1.1494x; 1.1494x over previous
"""Trainium2 Bass kernel for nn_DS_Attention_7636451852327.

Data-parallel over batch: 32 batches -> 8 NeuronCores, 4 batches (2048 tokens)
per core, 16 token-tiles of 128.

Host-side prep: q/v shipped pre-transposed ([512, T] fp16) so the QKV matmul
lhsT tiles are direct DMA loads (no on-device cast / PE transpose / PSUM
copy).  lin_w rows are permuted so the attention output is written in
(h, d, q) order (keeps the normalization multiply in DVE 2x mode), and the
output bias is folded into the final matmul via an appended ones-row.

Engine split (vertical, by head): DVE runs heads [0, HD) end-to-end plus the
front-end (products/d-sum/rowsum) of the Pool-side heads [HD, 8); the Pool
(GPSIMD) engine runs the back-end of those heads (PV products, k-sum tree,
normalize-by-divide, corrections). Pool only ever consumes DVE-produced
data -- DVE never waits on Pool mid-tile -- and every tile crossing the
engine boundary is double-buffered, so the two engines pipeline cleanly
about half a tile apart.  ACT does PSUM evictions and the exp()s.
"""
import os as _os
import numpy as np
from contextlib import ExitStack

import concourse.bass as bass
import concourse.mybir as mybir
import concourse.tile as tile
from concourse import bacc
from concourse.bass_utils import run_bass_kernel_spmd
from concourse.masks import make_identity

hp = mybir.dt.float16
f32 = mybir.dt.float32
AL = mybir.AluOpType
AX = mybir.AxisListType
AF = mybir.ActivationFunctionType

P = 128
H = 8
QJ = KJ = 24
D = 6
NQK = QJ * D              # 144
NVA = H * NQK             # 1152
NP = QJ * KJ              # 576 (q,k) pairs per head
DH = 1176                 # 147*8
D_MODEL = 512
W_TOT = 3 * NVA + H * 3   # 3480
B = 32
N = 512
N_CORES = 8
TT = (B // N_CORES) * N // P   # 16 token tiles per core

# custom-weighting chain levels: dst col range <- src col range (per head)
CH_LEVELS = (((6, 7), (3, 4)), ((9, 10), (6, 7)),
             ((12, 15), (9, 10)), ((15, 18), (12, 15)))

QKV_CHUNKS = [
    (0, 512, 0), (512, 1024, 0), (1024, 1152, 0),
    (1152, 1664, 1), (1664, 2176, 1), (2176, 2304, 1),
    (2304, 2816, 2), (2816, 3328, 2), (3328, 3480, 2),
]


def _cfg(name, default):
    v = _os.environ.get(name)
    return int(v) if v else default

HD = _cfg("HD", 5)        # DVE-side head count; Pool back-end owns the rest
HP = H - HD


def build_program(tt=TT, inner_repeat=1):
    nc = bacc.Bacc("TRN2", target_bir_lowering=False, debug=False)
    T = tt * P
    qT_dram = nc.dram_tensor("qT", [D_MODEL, T], hp, kind="ExternalInput").ap()
    vT_dram = nc.dram_tensor("vT", [D_MODEL, T], hp, kind="ExternalInput").ap()
    wcat_dram = nc.dram_tensor("w_cat", [D_MODEL, W_TOT], hp, kind="ExternalInput").ap()
    lw_dram = nc.dram_tensor("lin_w", [1184, D_MODEL], hp, kind="ExternalInput").ap()
    out_dram = nc.dram_tensor("out", [T, D_MODEL], f32, kind="ExternalOutput").ap()

    dve, pool, act = nc.vector, nc.gpsimd, nc.scalar

    with tile.TileContext(nc) as tc, ExitStack() as ctx:
        const = ctx.enter_context(tc.tile_pool(name="const", bufs=1))
        wpool = ctx.enter_context(tc.tile_pool(name="wpool", bufs=1))
        io = ctx.enter_context(tc.tile_pool(name="io", bufs=2))
        qkv = ctx.enter_context(tc.tile_pool(name="qkv", bufs=1))
        vab = ctx.enter_context(tc.tile_pool(name="vab", bufs=2))
        vpt = ctx.enter_context(tc.tile_pool(name="vpt", bufs=2))
        bigD = ctx.enter_context(tc.tile_pool(name="bigD", bufs=1))
        bigP = ctx.enter_context(tc.tile_pool(name="bigP", bufs=2))
        poolP = ctx.enter_context(tc.tile_pool(name="poolP", bufs=1))
        small = ctx.enter_context(tc.tile_pool(name="small", bufs=1))
        smx = ctx.enter_context(tc.tile_pool(name="smx", bufs=2))
        vt = ctx.enter_context(tc.tile_pool(name="vt", bufs=1))
        outp = ctx.enter_context(tc.tile_pool(name="outp", bufs=2))
        ps_t = ctx.enter_context(tc.tile_pool(name="ps_t", bufs=2, space="PSUM"))
        ps_mm = ctx.enter_context(tc.tile_pool(name="ps_mm", bufs=2, space="PSUM"))
        ps_out = ctx.enter_context(tc.tile_pool(name="ps_out", bufs=2, space="PSUM"))

        ident = const.tile([P, P], hp, tag="ident")
        make_identity(nc, ident[:])
        wcat = []
        for k in range(4):
            wk = wpool.tile([P, W_TOT], hp, tag=f"wcat{k}")
            nc.sync.dma_start(wk[:], wcat_dram[k * P:(k + 1) * P, :])
            wcat.append(wk)
        lw = []
        for k in range(10):
            rows = min(P, DH - k * P)
            if k == 9:
                rows += 1  # bias row
            lwk = wpool.tile([P, D_MODEL], hp, tag=f"lw{k}")
            nc.sync.dma_start(lwk[:rows, :], lw_dram[k * P:k * P + rows, :])
            lw.append((lwk, rows))

        for it in range(tt):
          for _rep in range(inner_repeat):
            # ---- input tiles: direct transposed fp16 loads ----
            xq, xv = [], []
            for src, dst, nm in ((qT_dram, xq, "q"), (vT_dram, xv, "v")):
                for k in range(4):
                    xk = io.tile([P, P], hp, tag=f"x{nm}{k}")
                    nc.sync.dma_start(xk[:], src[k * P:(k + 1) * P, it * P:(it + 1) * P])
                    dst.append(xk)

            # ---- QKV projection (token-major) ----
            qa_all = qkv.tile([P, NVA], hp, tag="qa_all")
            ka_all = qkv.tile([P, NVA], hp, tag="ka_all")
            va_all = vab.tile([P, NVA], hp, tag="va_all")
            vptok = vpt.tile([P, DH + 1], hp, tag="vptok")
            pool.memset(vptok[:, DH:DH + 1], 1.0)  # ones col -> bias row of v'^T
            for (c0, c1, kind) in QKV_CHUNKS:
                w_n = c1 - c0
                pmm = ps_mm.tile([P, 512], f32, tag="pmm")
                lhs_tiles = xv if kind == 2 else xq
                for k in range(4):
                    nc.tensor.matmul(pmm[:, :w_n], lhs_tiles[k][:], wcat[k][:, c0:c1],
                                     start=(k == 0), stop=(k == 3))
                if kind == 0:
                    act.copy(qa_all[:, c0:c1], pmm[:, :w_n])
                elif kind == 1:
                    act.copy(ka_all[:, c0 - NVA:c1 - NVA], pmm[:, :w_n])
                else:
                    v0, v1 = c0 - 2 * NVA, c1 - 2 * NVA
                    if v1 <= NVA:
                        act.copy(va_all[:, v0:v1], pmm[:, :w_n])
                    else:
                        act.copy(va_all[:, v0:NVA], pmm[:, :NVA - v0])
                        vp = pmm[:, NVA - v0:w_n].rearrange("p (h c) -> p h c", h=H)
                        vp_dst = vptok[:, :DH].rearrange("p (h c) -> p h c", h=H)[:, :, :3]
                        act.copy(vp_dst, vp)

            qa_v = qa_all[:].rearrange("p (h q d) -> p h q d", h=H, q=QJ)
            ka_v = ka_all[:].rearrange("p (h k d) -> p h k d", h=H, k=KJ)
            va_v = va_all[:].rearrange("p (h d k) -> p h d k", h=H, d=D)
            att_all = vptok[:, :DH].rearrange("p (h c) -> p h c", h=H)[:, :, 3:]

            # ---- per-side tiles ----
            # D side: p1 groups + half-k p2 share pbigD; P side mirrors, 2 bufs.
            CD = min(HD, 3)                     # p1 group capacity (heads)
            CP = min(HP, 2)
            pbigD = bigD.tile([P, CD * NP * D], hp, tag="pbigD")
            pbigP = bigP.tile([P, CP * NP * D], hp, tag="pbigP")
            s2aD = bigD.tile([P, HD * NP * 2], hp, tag="s2aD")
            s2aP = bigD.tile([P, HP * NP * 2], hp, tag="s2aP")
            eaD = bigD.tile([P, HD * NP], hp, tag="eaD")
            ebD = bigD.tile([P, HD * NP], hp, tag="ebD")
            eD0 = bigD.tile([P, HD * QJ * 12], hp, tag="eD0")
            eD1 = bigD.tile([P, HD * QJ * 12], hp, tag="eD1")
            eaP = bigD.tile([P, HP * NP], hp, tag="eaP")
            ebP = bigD.tile([P, HP * NP], hp, tag="ebP")
            eP0 = bigP.tile([P, HP * QJ * 12], hp, tag="eP0")
            eP1 = bigP.tile([P, HP * QJ * 12], hp, tag="eP1")
            t6P = poolP.tile([P, HP * NQK * 6], hp, tag="t6P")
            t2P = poolP.tile([P, HP * NQK * 2], hp, tag="t2P")
            a0P = poolP.tile([P, HP * NQK], hp, tag="a0P")
            a0Pb = poolP.tile([P, HP * NQK], hp, tag="a0Pb")
            ctP = poolP.tile([P, HP * D * D], hp, tag="ctP")
            r12D = small.tile([P, HD * QJ * 12], hp, tag="r12D")
            r6D = small.tile([P, HD * QJ * 6], hp, tag="r6D")
            r2D = small.tile([P, HD * QJ * 2], hp, tag="r2D")
            sD = small.tile([P, HD * QJ], f32, tag="sD")
            r16D = small.tile([P, HD * QJ], hp, tag="r16D")
            r12P = small.tile([P, HP * QJ * 12], hp, tag="r12P")
            r6P = small.tile([P, HP * QJ * 6], hp, tag="r6P")
            r2P = small.tile([P, HP * QJ * 2], hp, tag="r2P")
            sP = small.tile([P, HP * QJ], f32, tag="sP")
            rP16 = smx.tile([P, HP * QJ], hp, tag="rP16")
            a0D = small.tile([P, HD * NQK], hp, tag="a0D")
            a0Db = small.tile([P, HD * NQK], hp, tag="a0Db")
            ctD = small.tile([P, HD * D * D], hp, tag="ctD")
            uD = small.tile([P, HD * QJ], hp, tag="uD")
            u2D = small.tile([P, HD * QJ], hp, tag="u2D")
            duD = small.tile([P, HD * QJ], hp, tag="duD")
            uP = small.tile([P, HP * QJ], hp, tag="uP")
            u2P = small.tile([P, HP * QJ], hp, tag="u2P")
            duP = smx.tile([P, HP * QJ], hp, tag="duP")
            tmp8D = small.tile([P, H * 3], hp, tag="tmp8D")
            tmp8P = poolP.tile([P, H * 3], hp, tag="tmp8P")

            def p1_s2a(pb, s2t, g0, g1, o):
                """products+d-sum pair lanes for global heads [g0,g1) at
                buffer head-offset o of pbig, side s2a offset rel to side."""
                nh = g1 - g0
                p1v = pb[:, o * NP * D:(o + nh) * NP * D].rearrange(
                    "p (h q k d) -> p h q k d", h=nh, q=QJ, k=KJ)
                p1f = pb[:, o * NP * D:(o + nh) * NP * D].rearrange(
                    "p (h pr d) -> p h pr d", h=nh, pr=NP)
                qa_b = qa_v[:, g0:g1].unsqueeze(3).broadcast_to([P, nh, QJ, KJ, D])
                ka_b = ka_v[:, g0:g1].unsqueeze(2).broadcast_to([P, nh, QJ, KJ, D])
                dve.tensor_tensor(p1v, qa_b, ka_b, AL.mult)
                rel = g0 - (0 if s2t is s2aD else HD)
                s2v = s2t[:, rel * NP * 2:(rel + nh) * NP * 2].rearrange(
                    "p (h pr e) -> p h pr e", h=nh, pr=NP)
                dve.tensor_tensor(s2v, p1f[:, :, :, 0:2], p1f[:, :, :, 2:4], AL.add)
                dve.tensor_tensor(s2v, s2v, p1f[:, :, :, 4:6], AL.add)

            # D-side p1 groups, then P-side (all on DVE), exps interleave on ACT
            p1_s2a(pbigD, s2aD, 0, CD, 0)
            s2Dv = s2aD[:].rearrange("p (h pr e) -> p h pr e", h=HD, pr=NP)
            act.activation(eaD[:, :CD * NP], s2Dv[:, :CD, :, 0], AF.Exp)
            act.activation(ebD[:, :CD * NP], s2Dv[:, :CD, :, 1], AF.Exp)
            if HD > CD:
                p1_s2a(pbigD, s2aD, CD, HD, 0)
                act.activation(eaD[:, CD * NP:], s2Dv[:, CD:, :, 0], AF.Exp)
                act.activation(ebD[:, CD * NP:], s2Dv[:, CD:, :, 1], AF.Exp)
            s2Pv = s2aP[:].rearrange("p (h pr e) -> p h pr e", h=HP, pr=NP)
            p1_s2a(pbigP, s2aP, HD, HD + CP, 0)
            act.activation(eaP[:, :CP * NP], s2Pv[:, :CP, :, 0], AF.Exp)
            act.activation(ebP[:, :CP * NP], s2Pv[:, :CP, :, 1], AF.Exp)
            if HP > CP:
                p1_s2a(pbigP, s2aP, HD + CP, H, 0)
                act.activation(eaP[:, CP * NP:], s2Pv[:, CP:, :, 0], AF.Exp)
                act.activation(ebP[:, CP * NP:], s2Pv[:, CP:, :, 1], AF.Exp)

            # e = ea*eb (both sides on DVE), stored as k-halves
            for (ea_t, eb_t, e0, e1, nh) in ((eaD, ebD, eD0, eD1, HD),
                                             (eaP, ebP, eP0, eP1, HP)):
                eav = ea_t[:].rearrange("p (h q k) -> p h q k", h=nh, q=QJ)
                ebv = eb_t[:].rearrange("p (h q k) -> p h q k", h=nh, q=QJ)
                for kh, et in ((0, e0), (1, e1)):
                    dve.tensor_tensor(
                        et[:].rearrange("p (h q k) -> p h q k", h=nh, q=QJ),
                        eav[:, :, :, kh * 12:(kh + 1) * 12],
                        ebv[:, :, :, kh * 12:(kh + 1) * 12], AL.mult)

            # rowsums (both sides on DVE): 24 -> 12 -> 6 -> 2 -> 1
            for (e0, e1, r12t, r6t, r2t, st, nh) in (
                    (eD0, eD1, r12D, r6D, r2D, sD, HD),
                    (eP0, eP1, r12P, r6P, r2P, sP, HP)):
                fq = nh * QJ
                r12v = r12t[:].rearrange("p (f k) -> p f k", f=fq)
                r6v = r6t[:].rearrange("p (f k) -> p f k", f=fq)
                r2v = r2t[:].rearrange("p (f k) -> p f k", f=fq)
                dve.tensor_tensor(r12t[:], e0[:], e1[:], AL.add)
                dve.tensor_tensor(r6v, r12v[:, :, 0:6], r12v[:, :, 6:12], AL.add)
                dve.tensor_tensor(r2v, r6v[:, :, 0:2], r6v[:, :, 2:4], AL.add)
                dve.tensor_tensor(r2v, r2v, r6v[:, :, 4:6], AL.add)
                with nc.allow_low_precision(reason="fp16 attention"):
                    dve.tensor_tensor(st[:], r2v[:, :, 0], r2v[:, :, 1], AL.add)
            with nc.allow_low_precision(reason="fp16 attention"):
                dve.reciprocal(r16D[:], sD[:])
                dve.reciprocal(rP16[:], sP[:])

            eD0v = eD0[:].rearrange("p (h q k) -> p h q k", h=HD, q=QJ)
            eD1v = eD1[:].rearrange("p (h q k) -> p h q k", h=HD, q=QJ)
            eP0v = eP0[:].rearrange("p (h q k) -> p h q k", h=HP, q=QJ)
            eP1v = eP1[:].rearrange("p (h q k) -> p h q k", h=HP, q=QJ)
            rDv = r16D[:].rearrange("p (h q) -> p h q", h=HD)
            rPv = rP16[:].rearrange("p (h q) -> p h q", h=HP)

            # u = p[:,0]: D side e0*r, P side e0/s (both DVE); u2 = chained copy
            dve.tensor_tensor(uD[:].rearrange("p (h q) -> p h q", h=HD),
                              eD0v[:, :, :, 0], rDv, AL.mult)
            dve.tensor_tensor(uP[:].rearrange("p (h q) -> p h q", h=HP),
                              eP0v[:, :, :, 0], rPv, AL.mult)
            act.copy(u2D[:], uD[:])
            act.copy(u2P[:], uP[:])

            # chains: D-side e-row0 + u2D + u2P on DVE; P-side e-row0 on Pool
            def chains(eng, resolve, nh, toff, tmp):
                t3 = tmp[:, toff * 3:(toff + nh) * 3].rearrange("p (h c) -> p h c", h=nh)
                for dsl, ssl in CH_LEVELS:
                    nd = dsl[1] - dsl[0]
                    dst = resolve(dsl[0], dsl[1])
                    src = resolve(ssl[0], ssl[1])
                    if ssl[1] - ssl[0] < nd:
                        src = src.broadcast_to([P, nh, nd])
                    eng.tensor_tensor(t3[:, :, :nd], dst, src, AL.add)
                    eng.tensor_scalar_mul(dst, t3[:, :, :nd], 0.5)

            def e_resolver(e0v, e1v):
                def resolve(c0, c1):
                    if c1 <= 12:
                        return e0v[:, :, 0, c0:c1]
                    return e1v[:, :, 0, c0 - 12:c1 - 12]
                return resolve

            def flat_resolver(v):
                return lambda c0, c1: v[:, :, c0:c1]

            chains(dve, e_resolver(eD0v, eD1v), HD, 0, tmp8D)
            chains(dve, flat_resolver(u2D[:].rearrange("p (h q) -> p h q", h=HD)), HD, 0, tmp8D)
            chains(dve, flat_resolver(u2P[:].rearrange("p (h q) -> p h q", h=HP)), HP, HD, tmp8D)
            chains(pool, e_resolver(eP0v, eP1v), HP, 0, tmp8P)  # after rowsum-P on DVE
            dve.tensor_tensor(duD[:], u2D[:], uD[:], AL.subtract)
            dve.tensor_tensor(duP[:], u2P[:], uP[:], AL.subtract)

            # ---- back-end per side: p2 (half-k) -> tree -> att ----
            def backend(eng, pb, e0v, e1v, g0, g1, t6t, t2t, a0t, a0bt):
                nh = g1 - g0
                for kh, a0o in ((0, a0t), (1, a0bt)):
                    ehv = e0v if kh == 0 else e1v
                    p2v = pb[:, :nh * NQK * 12].rearrange(
                        "p (h d q k) -> p h d q k", h=nh, d=D, q=QJ)
                    if eng is pool:
                        # Pool TT is limited to 3 free dims: per-head ops with
                        # integer head indexing.
                        for hh in range(nh):
                            e_b = ehv[:, hh].unsqueeze(1).broadcast_to([P, D, QJ, 12])
                            va_b = va_v[:, g0 + hh, :, kh * 12:(kh + 1) * 12
                                        ].unsqueeze(2).broadcast_to([P, D, QJ, 12])
                            eng.tensor_tensor(p2v[:, hh], e_b, va_b, AL.mult)
                    else:
                        e_b = ehv.unsqueeze(2).broadcast_to([P, nh, D, QJ, 12])
                        va_b = va_v[:, g0:g1, :, kh * 12:(kh + 1) * 12].unsqueeze(
                            3).broadcast_to([P, nh, D, QJ, 12])
                        eng.tensor_tensor(p2v, e_b, va_b, AL.mult)
                    p2f = pb[:, :nh * NQK * 12].rearrange(
                        "p (h f k) -> p h f k", h=nh, f=NQK)
                    t6v = t6t[:, :nh * NQK * 6].rearrange("p (h f k) -> p h f k", h=nh, f=NQK)
                    t2v = t2t[:, :nh * NQK * 2].rearrange("p (h f k) -> p h f k", h=nh, f=NQK)
                    eng.tensor_tensor(t6v, p2f[:, :, :, 0:6], p2f[:, :, :, 6:12], AL.add)
                    eng.tensor_tensor(t2v, t6v[:, :, :, 0:2], t6v[:, :, :, 2:4], AL.add)
                    eng.tensor_tensor(t2v, t2v, t6v[:, :, :, 4:6], AL.add)
                    eng.tensor_tensor(a0o[:].rearrange("p (h f) -> p h f", h=nh),
                                      t2v[:, :, :, 0], t2v[:, :, :, 1], AL.add)
                eng.tensor_tensor(a0t[:], a0t[:], a0bt[:], AL.add)

            # D side back-end on DVE
            backend(dve, pbigD, eD0v, eD1v, 0, HD, s2aD, r12D, a0D, a0Db)
            a0Dv = a0D[:].rearrange("p (h d q) -> p h d q", h=HD, d=D)
            attD = att_all[:, :HD].rearrange("p h (d q) -> p h d q", d=D)
            r_bD = rDv.unsqueeze(2).broadcast_to([P, HD, D, QJ])
            dve.tensor_tensor(attD, a0Dv, r_bD, AL.mult)
            duDv = duD[:].rearrange("p (h q) -> p h q", h=HD)
            va0D = va_v[:, :HD, :, 0]
            ctDv = ctD[:].rearrange("p (h d q) -> p h d q", h=HD, d=D)
            for qsl, n_q in (((12, 18, 1), 6), ((6, 10, 3), 2)):
                du_b = duDv[:, :, qsl[0]:qsl[1]:qsl[2]].unsqueeze(2).broadcast_to(
                    [P, HD, D, n_q])
                va0_b = va0D.unsqueeze(3).broadcast_to([P, HD, D, n_q])
                dve.tensor_tensor(ctDv[:, :, :, :n_q], du_b, va0_b, AL.mult)
                dve.tensor_tensor(attD[:, :, :, qsl[0]:qsl[1]:qsl[2]],
                                  attD[:, :, :, qsl[0]:qsl[1]:qsl[2]],
                                  ctDv[:, :, :, :n_q], AL.add)

            # P side back-end on Pool (consumes DVE-produced eP/duP/sP)
            backend(pool, pbigP, eP0v, eP1v, HD, H, t6P, t2P, a0P, a0Pb)
            a0Pv = a0P[:].rearrange("p (h d q) -> p h d q", h=HP, d=D)
            attP = att_all[:, HD:].rearrange("p h (d q) -> p h d q", d=D)
            r_bP = rPv.unsqueeze(2).broadcast_to([P, HP, D, QJ])
            pool.tensor_tensor(attP, a0Pv, r_bP, AL.mult)
            duPv = duP[:].rearrange("p (h q) -> p h q", h=HP)
            va0P = va_v[:, HD:, :, 0]
            ctPv = ctP[:].rearrange("p (h d q) -> p h d q", h=HP, d=D)
            for qsl, n_q in (((12, 18, 1), 6), ((6, 10, 3), 2)):
                du_b = duPv[:, :, qsl[0]:qsl[1]:qsl[2]].unsqueeze(2).broadcast_to(
                    [P, HP, D, n_q])
                va0_b = va0P.unsqueeze(3).broadcast_to([P, HP, D, n_q])
                pool.tensor_tensor(ctPv[:, :, :, :n_q], du_b, va0_b, AL.mult)
                pool.tensor_tensor(attP[:, :, :, qsl[0]:qsl[1]:qsl[2]],
                                   attP[:, :, :, qsl[0]:qsl[1]:qsl[2]],
                                   ctPv[:, :, :, :n_q], AL.add)

            # ---- output transposes + final matmul (bias folded via ones row) ----
            vT = []
            for k in range(10):
                cols = min(P, DH + 1 - k * P)
                pst2 = ps_t.tile([P, P], hp, tag="pst2")
                nc.tensor.transpose(pst2[:cols, :], vptok[:, k * P:k * P + cols], ident[:])
                vk = vt.tile([P, P], hp, tag=f"vT{k}")
                act.copy(vk[:cols, :], pst2[:cols, :])
                vT.append((vk, cols))
            pout = ps_out.tile([P, D_MODEL], f32, tag="pout")
            for k in range(10):
                vk, rows = vT[k]
                lwk, rows2 = lw[k]
                nc.tensor.matmul(pout[:], vk[:rows, :], lwk[:rows, :],
                                 start=(k == 0), stop=(k == 9))
            out_sb = outp.tile([P, D_MODEL], f32, tag="out_sb")
            act.copy(out_sb[:], pout[:])
            nc.sync.dma_start(out_dram[it * P:(it + 1) * P, :], out_sb[:])

    nc.compile()
    return nc


def prep_weights(qk_w, v_w, lin_w, lin_b):
    scale = np.float32(1.0 / np.sqrt(6.0))
    wq = np.asarray(qk_w[:, :DH], dtype=np.float32).reshape(D_MODEL, H, 147)
    wk = np.asarray(qk_w[:, DH:], dtype=np.float32).reshape(D_MODEL, H, 147)
    wv = np.asarray(v_w, dtype=np.float32).reshape(D_MODEL, H, 147)
    wq_p = (wq[:, :, 3:] * scale).reshape(D_MODEL, H * NQK)
    wk_p = wk[:, :, 3:].reshape(D_MODEL, H * NQK)
    wv_att = wv[:, :, 3:].reshape(D_MODEL, H, KJ, D).transpose(0, 1, 3, 2).reshape(D_MODEL, H * NQK)
    wv_pass = wv[:, :, :3].reshape(D_MODEL, H * 3)
    w_cat = np.ascontiguousarray(
        np.concatenate([wq_p, wk_p, wv_att, wv_pass], axis=1)).astype(np.float16)
    # lin_w rows permuted to the (h, [pass3, d*24+q]) vptok layout + bias row
    lwr = np.asarray(lin_w, dtype=np.float32).reshape(H, 147, D_MODEL)
    att = lwr[:, 3:, :].reshape(H, QJ, D, D_MODEL).transpose(0, 2, 1, 3).reshape(H, NQK, D_MODEL)
    lw_p = np.concatenate([lwr[:, :3, :], att], axis=1).reshape(DH, D_MODEL)
    lw_aug = np.zeros((1184, D_MODEL), dtype=np.float32)
    lw_aug[:DH] = lw_p
    lw_aug[DH] = np.asarray(lin_b, dtype=np.float32)
    return w_cat, np.ascontiguousarray(lw_aug).astype(np.float16)


def make_in_maps(query, value, qk_w, v_w, lin_w, lin_b):
    w_cat, lw_aug = prep_weights(qk_w, v_w, lin_w, lin_b)
    q = np.asarray(query, dtype=np.float32)
    v = np.asarray(value, dtype=np.float32)
    bpc = B // N_CORES
    in_maps = []
    for c in range(N_CORES):
        qc = q[c * bpc:(c + 1) * bpc].reshape(-1, D_MODEL).T
        vc = v[c * bpc:(c + 1) * bpc].reshape(-1, D_MODEL).T
        in_maps.append({
            "qT": np.ascontiguousarray(qc).astype(np.float16),
            "vT": np.ascontiguousarray(vc).astype(np.float16),
            "w_cat": w_cat,
            "lin_w": lw_aug,
        })
    return in_maps


_CACHED_NC = None


def _get_nc():
    global _CACHED_NC
    if _CACHED_NC is None:
        _CACHED_NC = build_program(TT)
    return _CACHED_NC


def kernel(query, key, value, qk_w, v_w, lin_w, lin_b, _want_results=False, **_ignored):
    """Full-input kernel: shards batch over 8 cores, returns full output."""
    in_maps = make_in_maps(query, value, qk_w, v_w, lin_w, lin_b)
    nc = _get_nc()
    bpc = B // N_CORES
    res = run_bass_kernel_spmd(nc, in_maps, core_ids=list(range(N_CORES)))
    out = np.empty((B, N, D_MODEL), dtype=np.float32)
    for c in range(N_CORES):
        out[c * bpc:(c + 1) * bpc] = res.results[c]["out"].reshape(bpc, N, D_MODEL)
    if _want_results:
        return out, res
    return out


# revision 12
# speedup vs baseline: 1.2636x; 1.0994x over previous
"""Trainium2 Bass kernel for nn_DS_Attention_7636451852327.

Data-parallel over batch: 32 batches -> 8 NeuronCores, 4 batches (2048 tokens)
per core, 16 token-tiles of 128.

Host-side prep: q/v shipped pre-transposed ([512, T] fp16) so the QKV matmul
lhsT tiles are direct DMA loads (no on-device cast / PE transpose / PSUM
copy).  lin_w rows are permuted so the attention output is written in
(h, d, q) order (keeps the normalization multiply in DVE 2x mode), and the
output bias is folded into the final matmul via an appended ones-row.

Engine split (vertical, by head): DVE runs heads [0, HD) end-to-end plus the
front-end (products/d-sum/rowsum) of the Pool-side heads [HD, 8); the Pool
(GPSIMD) engine runs the back-end of those heads (PV products, k-sum tree,
normalize-by-divide, corrections). Pool only ever consumes DVE-produced
data -- DVE never waits on Pool mid-tile -- and every tile crossing the
engine boundary is double-buffered, so the two engines pipeline cleanly
about half a tile apart.  ACT does PSUM evictions and the exp()s.
"""
import os as _os
import numpy as np
from contextlib import ExitStack

import concourse.bass as bass
import concourse.mybir as mybir
import concourse.tile as tile
from concourse import bacc
from concourse.bass_utils import run_bass_kernel_spmd
from concourse.masks import make_identity

hp = mybir.dt.float16
f32 = mybir.dt.float32
AL = mybir.AluOpType
AX = mybir.AxisListType
AF = mybir.ActivationFunctionType

P = 128
H = 8
QJ = KJ = 24
D = 6
NQK = QJ * D              # 144
NVA = H * NQK             # 1152
NP = QJ * KJ              # 576 (q,k) pairs per head
DH = 1176                 # 147*8
D_MODEL = 512
W_TOT = 3 * NVA + H * 3   # 3480
B = 32
N = 512
N_CORES = 8
TT = (B // N_CORES) * N // P   # 16 token tiles per core

# custom-weighting chain levels: dst col range <- src col range (per head)
CH_LEVELS = (((6, 7), (3, 4)), ((9, 10), (6, 7)),
             ((12, 15), (9, 10)), ((15, 18), (12, 15)))

QKV_CHUNKS = [
    (0, 512, 0), (512, 1024, 0), (1024, 1152, 0),
    (1152, 1664, 1), (1664, 2176, 1), (2176, 2304, 1),
    (2304, 2816, 2), (2816, 3328, 2), (3328, 3480, 2),
]


def _cfg(name, default):
    v = _os.environ.get(name)
    return int(v) if v else default

HD = _cfg("HD", 5)        # DVE-side head count; Pool back-end owns the rest
HP = H - HD


def build_program(tt=TT, inner_repeat=1):
    nc = bacc.Bacc("TRN2", target_bir_lowering=False, debug=False)
    T = tt * P
    qT_dram = nc.dram_tensor("qT", [D_MODEL, T], hp, kind="ExternalInput").ap()
    vT_dram = nc.dram_tensor("vT", [D_MODEL, T], hp, kind="ExternalInput").ap()
    wcat_dram = nc.dram_tensor("w_cat", [D_MODEL, W_TOT], hp, kind="ExternalInput").ap()
    lw_dram = nc.dram_tensor("lin_w", [1184, D_MODEL], hp, kind="ExternalInput").ap()
    out_dram = nc.dram_tensor("out", [T, D_MODEL], f32, kind="ExternalOutput").ap()

    dve, pool, act = nc.vector, nc.gpsimd, nc.scalar

    with tile.TileContext(nc) as tc, ExitStack() as ctx:
        const = ctx.enter_context(tc.tile_pool(name="const", bufs=1))
        wpool = ctx.enter_context(tc.tile_pool(name="wpool", bufs=1))
        io = ctx.enter_context(tc.tile_pool(name="io", bufs=2))
        qkv = ctx.enter_context(tc.tile_pool(name="qkv", bufs=1))
        vab = ctx.enter_context(tc.tile_pool(name="vab", bufs=2))
        vpt = ctx.enter_context(tc.tile_pool(name="vpt", bufs=2))
        bigD = ctx.enter_context(tc.tile_pool(name="bigD", bufs=1))
        bigP = ctx.enter_context(tc.tile_pool(name="bigP", bufs=2))
        poolP = ctx.enter_context(tc.tile_pool(name="poolP", bufs=1))
        small = ctx.enter_context(tc.tile_pool(name="small", bufs=1))
        smx = ctx.enter_context(tc.tile_pool(name="smx", bufs=2))
        vt = ctx.enter_context(tc.tile_pool(name="vt", bufs=1))
        outp = ctx.enter_context(tc.tile_pool(name="outp", bufs=2))
        ps_t = ctx.enter_context(tc.tile_pool(name="ps_t", bufs=2, space="PSUM"))
        ps_mm = ctx.enter_context(tc.tile_pool(name="ps_mm", bufs=2, space="PSUM"))
        ps_out = ctx.enter_context(tc.tile_pool(name="ps_out", bufs=2, space="PSUM"))

        ident = const.tile([P, P], hp, tag="ident")
        make_identity(nc, ident[:])
        wcat = []
        for k in range(4):
            wk = wpool.tile([P, W_TOT], hp, tag=f"wcat{k}")
            nc.sync.dma_start(wk[:], wcat_dram[k * P:(k + 1) * P, :])
            wcat.append(wk)
        lw = []
        for k in range(10):
            rows = min(P, DH - k * P)
            if k == 9:
                rows += 1  # bias row
            lwk = wpool.tile([P, D_MODEL], hp, tag=f"lw{k}")
            nc.sync.dma_start(lwk[:rows, :], lw_dram[k * P:k * P + rows, :])
            lw.append((lwk, rows))

        pending = None
        for it in range(tt):
          for _rep in range(inner_repeat):
            # ---- input tiles: direct transposed fp16 loads ----
            xq, xv = [], []
            for src, dst, nm in ((qT_dram, xq, "q"), (vT_dram, xv, "v")):
                for k in range(4):
                    xk = io.tile([P, P], hp, tag=f"x{nm}{k}")
                    nc.sync.dma_start(xk[:], src[k * P:(k + 1) * P, it * P:(it + 1) * P])
                    dst.append(xk)

            # ---- QKV projection: q/k chunks first (ACT copies feed DVE) ----
            qa_all = qkv.tile([P, NVA], hp, tag="qa_all")
            ka_all = qkv.tile([P, NVA], hp, tag="ka_all")
            va_all = vab.tile([P, NVA], hp, tag="va_all")
            vptok = vpt.tile([P, DH + 1], hp, tag="vptok")
            pool.memset(vptok[:, DH:DH + 1], 1.0)  # ones col -> bias row of v'^T

            def qkv_chunk(c0, c1, kind):
                w_n = c1 - c0
                pmm = ps_mm.tile([P, 512], f32, tag="pmm")
                lhs_tiles = xv if kind == 2 else xq
                for k in range(4):
                    nc.tensor.matmul(pmm[:, :w_n], lhs_tiles[k][:], wcat[k][:, c0:c1],
                                     start=(k == 0), stop=(k == 3))
                if kind == 0:
                    act.copy(qa_all[:, c0:c1], pmm[:, :w_n])
                elif kind == 1:
                    act.copy(ka_all[:, c0 - NVA:c1 - NVA], pmm[:, :w_n])
                else:
                    v0, v1 = c0 - 2 * NVA, c1 - 2 * NVA
                    if v1 <= NVA:
                        act.copy(va_all[:, v0:v1], pmm[:, :w_n])
                    else:
                        act.copy(va_all[:, v0:NVA], pmm[:, :NVA - v0])
                        vp = pmm[:, NVA - v0:w_n].rearrange("p (h c) -> p h c", h=H)
                        vp_dst = vptok[:, :DH].rearrange("p (h c) -> p h c", h=H)[:, :, :3]
                        act.copy(vp_dst, vp)

            for (c0, c1, kind) in QKV_CHUNKS[:6]:
                qkv_chunk(c0, c1, kind)

            qa_v = qa_all[:].rearrange("p (h q d) -> p h q d", h=H, q=QJ)
            ka_v = ka_all[:].rearrange("p (h k d) -> p h k d", h=H, k=KJ)
            va_v = va_all[:].rearrange("p (h d k) -> p h d k", h=H, d=D)
            att_all = vptok[:, :DH].rearrange("p (h c) -> p h c", h=H)[:, :, 3:]

            # epilogue(i-1) part 1: PE transposes of previous tile's v'
            if pending is not None:
                pvpt, pit = pending
                pvT = []
                for k in range(10):
                    cols = min(P, DH + 1 - k * P)
                    pst2 = ps_t.tile([P, P], hp, tag="pst2")
                    nc.tensor.transpose(pst2[:cols, :], pvpt[:, k * P:k * P + cols], ident[:])
                    pvT.append((pst2, cols))

            # ---- per-side tiles ----
            CD = min(HD, 3)                     # p1 group capacity (heads)
            CP = min(HP, 2)
            pbigD = bigD.tile([P, CD * NP * D], hp, tag="pbigD")
            pbigP = bigP.tile([P, CP * NP * D], hp, tag="pbigP")
            s2aD = bigD.tile([P, HD * NP * 2], hp, tag="s2aD")
            s2aP = bigD.tile([P, HP * NP * 2], hp, tag="s2aP")
            eaD = bigD.tile([P, HD * NP], hp, tag="eaD")
            ebD = bigD.tile([P, HD * NP], hp, tag="ebD")
            eD0 = bigD.tile([P, HD * QJ * 12], hp, tag="eD0")
            eD1 = bigD.tile([P, HD * QJ * 12], hp, tag="eD1")
            eaP = bigD.tile([P, HP * NP], hp, tag="eaP")
            ebP = bigD.tile([P, HP * NP], hp, tag="ebP")
            eP0 = bigP.tile([P, HP * QJ * 12], hp, tag="eP0")
            eP1 = bigP.tile([P, HP * QJ * 12], hp, tag="eP1")
            t6P = poolP.tile([P, HP * NQK * 6], hp, tag="t6P")
            t2P = poolP.tile([P, HP * NQK * 2], hp, tag="t2P")
            a0P = poolP.tile([P, HP * NQK], hp, tag="a0P")
            a0Pb = poolP.tile([P, HP * NQK], hp, tag="a0Pb")
            ctP = poolP.tile([P, HP * D * D], hp, tag="ctP")
            r12D = small.tile([P, HD * QJ * 12], hp, tag="r12D")
            r6D = small.tile([P, HD * QJ * 6], hp, tag="r6D")
            r2D = small.tile([P, HD * QJ * 2], hp, tag="r2D")
            sD = small.tile([P, HD * QJ], f32, tag="sD")
            r16D = small.tile([P, HD * QJ], hp, tag="r16D")
            r12P = small.tile([P, HP * QJ * 12], hp, tag="r12P")
            r6P = small.tile([P, HP * QJ * 6], hp, tag="r6P")
            r2P = small.tile([P, HP * QJ * 2], hp, tag="r2P")
            sP = small.tile([P, HP * QJ], f32, tag="sP")
            rP16 = smx.tile([P, HP * QJ], hp, tag="rP16")
            a0D = small.tile([P, HD * NQK], hp, tag="a0D")
            a0Db = small.tile([P, HD * NQK], hp, tag="a0Db")
            ctD = small.tile([P, HD * D * D], hp, tag="ctD")
            uD = small.tile([P, HD * QJ], hp, tag="uD")
            u2D = small.tile([P, HD * QJ], hp, tag="u2D")
            duD = small.tile([P, HD * QJ], hp, tag="duD")
            uP = small.tile([P, HP * QJ], hp, tag="uP")
            u2P = small.tile([P, HP * QJ], hp, tag="u2P")
            duP = smx.tile([P, HP * QJ], hp, tag="duP")
            tmp8D = small.tile([P, H * 3], hp, tag="tmp8D")
            tmp8P = poolP.tile([P, H * 3], hp, tag="tmp8P")

            s2Dv = s2aD[:].rearrange("p (h pr e) -> p h pr e", h=HD, pr=NP)
            s2Pv = s2aP[:].rearrange("p (h pr e) -> p h pr e", h=HP, pr=NP)

            def p1_s2a(pb, s2t, g0, g1, o):
                nh = g1 - g0
                p1v = pb[:, o * NP * D:(o + nh) * NP * D].rearrange(
                    "p (h q k d) -> p h q k d", h=nh, q=QJ, k=KJ)
                p1f = pb[:, o * NP * D:(o + nh) * NP * D].rearrange(
                    "p (h pr d) -> p h pr d", h=nh, pr=NP)
                qa_b = qa_v[:, g0:g1].unsqueeze(3).broadcast_to([P, nh, QJ, KJ, D])
                ka_b = ka_v[:, g0:g1].unsqueeze(2).broadcast_to([P, nh, QJ, KJ, D])
                dve.tensor_tensor(p1v, qa_b, ka_b, AL.mult)
                rel = g0 - (0 if s2t is s2aD else HD)
                s2v = s2t[:, rel * NP * 2:(rel + nh) * NP * 2].rearrange(
                    "p (h pr e) -> p h pr e", h=nh, pr=NP)
                dve.tensor_tensor(s2v, p1f[:, :, :, 0:2], p1f[:, :, :, 2:4], AL.add)
                dve.tensor_tensor(s2v, s2v, p1f[:, :, :, 4:6], AL.add)

            def emult(ea_t, eb_t, e0, e1, nh):
                eav = ea_t[:].rearrange("p (h q k) -> p h q k", h=nh, q=QJ)
                ebv = eb_t[:].rearrange("p (h q k) -> p h q k", h=nh, q=QJ)
                for kh, et in ((0, e0), (1, e1)):
                    dve.tensor_tensor(
                        et[:].rearrange("p (h q k) -> p h q k", h=nh, q=QJ),
                        eav[:, :, :, kh * 12:(kh + 1) * 12],
                        ebv[:, :, :, kh * 12:(kh + 1) * 12], AL.mult)

            def rowsum(e0, e1, r12t, r6t, r2t, st, nh):
                fq = nh * QJ
                r12v = r12t[:].rearrange("p (f k) -> p f k", f=fq)
                r6v = r6t[:].rearrange("p (f k) -> p f k", f=fq)
                r2v = r2t[:].rearrange("p (f k) -> p f k", f=fq)
                dve.tensor_tensor(r12t[:], e0[:], e1[:], AL.add)
                dve.tensor_tensor(r6v, r12v[:, :, 0:6], r12v[:, :, 6:12], AL.add)
                dve.tensor_tensor(r2v, r6v[:, :, 0:2], r6v[:, :, 2:4], AL.add)
                dve.tensor_tensor(r2v, r2v, r6v[:, :, 4:6], AL.add)
                with nc.allow_low_precision(reason="fp16 attention"):
                    dve.tensor_tensor(st[:], r2v[:, :, 0], r2v[:, :, 1], AL.add)

            def chains(eng, resolve, nh, toff, tmp):
                t3 = tmp[:, toff * 3:(toff + nh) * 3].rearrange("p (h c) -> p h c", h=nh)
                for dsl, ssl in CH_LEVELS:
                    nd = dsl[1] - dsl[0]
                    dst = resolve(dsl[0], dsl[1])
                    srcv = resolve(ssl[0], ssl[1])
                    if ssl[1] - ssl[0] < nd:
                        srcv = srcv.broadcast_to([P, nh, nd])
                    eng.tensor_tensor(t3[:, :, :nd], dst, srcv, AL.add)
                    eng.tensor_scalar_mul(dst, t3[:, :, :nd], 0.5)

            def e_resolver(e0v, e1v):
                def resolve(c0, c1):
                    if c1 <= 12:
                        return e0v[:, :, 0, c0:c1]
                    return e1v[:, :, 0, c0 - 12:c1 - 12]
                return resolve

            def flat_resolver(v):
                return lambda c0, c1: v[:, :, c0:c1]

            def backend(eng, pb, e0v, e1v, g0, g1, t6t, t2t, a0t, a0bt):
                nh = g1 - g0
                for kh, a0o in ((0, a0t), (1, a0bt)):
                    ehv = e0v if kh == 0 else e1v
                    p2v = pb[:, :nh * NQK * 12].rearrange(
                        "p (h d q k) -> p h d q k", h=nh, d=D, q=QJ)
                    if eng is pool:
                        for hh in range(nh):
                            e_b = ehv[:, hh].unsqueeze(1).broadcast_to([P, D, QJ, 12])
                            va_b = va_v[:, g0 + hh, :, kh * 12:(kh + 1) * 12
                                        ].unsqueeze(2).broadcast_to([P, D, QJ, 12])
                            eng.tensor_tensor(p2v[:, hh], e_b, va_b, AL.mult)
                    else:
                        e_b = ehv.unsqueeze(2).broadcast_to([P, nh, D, QJ, 12])
                        va_b = va_v[:, g0:g1, :, kh * 12:(kh + 1) * 12].unsqueeze(
                            3).broadcast_to([P, nh, D, QJ, 12])
                        eng.tensor_tensor(p2v, e_b, va_b, AL.mult)
                    p2f = pb[:, :nh * NQK * 12].rearrange(
                        "p (h f k) -> p h f k", h=nh, f=NQK)
                    t6v = t6t[:, :nh * NQK * 6].rearrange("p (h f k) -> p h f k", h=nh, f=NQK)
                    t2v = t2t[:, :nh * NQK * 2].rearrange("p (h f k) -> p h f k", h=nh, f=NQK)
                    eng.tensor_tensor(t6v, p2f[:, :, :, 0:6], p2f[:, :, :, 6:12], AL.add)
                    eng.tensor_tensor(t2v, t6v[:, :, :, 0:2], t6v[:, :, :, 2:4], AL.add)
                    eng.tensor_tensor(t2v, t2v, t6v[:, :, :, 4:6], AL.add)
                    eng.tensor_tensor(a0o[:].rearrange("p (h f) -> p h f", h=nh),
                                      t2v[:, :, :, 0], t2v[:, :, :, 1], AL.add)
                eng.tensor_tensor(a0t[:], a0t[:], a0bt[:], AL.add)

            def corr_att(eng, attv, a0v, r_b, du_t, va0, ct_t, nh):
                eng.tensor_tensor(attv, a0v, r_b, AL.mult)
                du_v = du_t[:].rearrange("p (h q) -> p h q", h=nh)
                ctv = ct_t[:].rearrange("p (h d q) -> p h d q", h=nh, d=D)
                for qsl, n_q in (((12, 18, 1), 6), ((6, 10, 3), 2)):
                    du_b = du_v[:, :, qsl[0]:qsl[1]:qsl[2]].unsqueeze(2).broadcast_to(
                        [P, nh, D, n_q])
                    va0_b = va0.unsqueeze(3).broadcast_to([P, nh, D, n_q])
                    eng.tensor_tensor(ctv[:, :, :, :n_q], du_b, va0_b, AL.mult)
                    eng.tensor_tensor(attv[:, :, :, qsl[0]:qsl[1]:qsl[2]],
                                      attv[:, :, :, qsl[0]:qsl[1]:qsl[2]],
                                      ctv[:, :, :, :n_q], AL.add)

            # --- DVE front-end: P side first (advances the Pool start gate) ---
            p1_s2a(pbigP, s2aP, HD, HD + CP, 0)
            act.activation(eaP[:, :CP * NP], s2Pv[:, :CP, :, 0], AF.Exp)
            act.activation(ebP[:, :CP * NP], s2Pv[:, :CP, :, 1], AF.Exp)
            if HP > CP:
                p1_s2a(pbigP, s2aP, HD + CP, H, 0)
                act.activation(eaP[:, CP * NP:], s2Pv[:, CP:, :, 0], AF.Exp)
                act.activation(ebP[:, CP * NP:], s2Pv[:, CP:, :, 1], AF.Exp)
            p1_s2a(pbigD, s2aD, 0, CD, 0)
            act.activation(eaD[:, :CD * NP], s2Dv[:, :CD, :, 0], AF.Exp)
            act.activation(ebD[:, :CD * NP], s2Dv[:, :CD, :, 1], AF.Exp)

            emult(eaP, ebP, eP0, eP1, HP)
            rowsum(eP0, eP1, r12P, r6P, r2P, sP, HP)
            with nc.allow_low_precision(reason="fp16 attention"):
                dve.reciprocal(rP16[:], sP[:])
            eP0v = eP0[:].rearrange("p (h q k) -> p h q k", h=HP, q=QJ)
            eP1v = eP1[:].rearrange("p (h q k) -> p h q k", h=HP, q=QJ)
            rPv = rP16[:].rearrange("p (h q) -> p h q", h=HP)
            dve.tensor_tensor(uP[:].rearrange("p (h q) -> p h q", h=HP),
                              eP0v[:, :, :, 0], rPv, AL.mult)
            act.copy(u2P[:], uP[:])
            chains(dve, flat_resolver(u2P[:].rearrange("p (h q) -> p h q", h=HP)),
                   HP, HD, tmp8D)
            dve.tensor_tensor(duP[:], u2P[:], uP[:], AL.subtract)
            # Pool may start its side now
            chains(pool, e_resolver(eP0v, eP1v), HP, 0, tmp8P)

            # --- DVE D-side front-end ---
            if HD > CD:
                p1_s2a(pbigD, s2aD, CD, HD, 0)
                act.activation(eaD[:, CD * NP:], s2Dv[:, CD:, :, 0], AF.Exp)
                act.activation(ebD[:, CD * NP:], s2Dv[:, CD:, :, 1], AF.Exp)

            # va chunks (ACT after the exps) -- Pool p2 needs them
            for (c0, c1, kind) in QKV_CHUNKS[6:]:
                qkv_chunk(c0, c1, kind)

            # epilogue(i-1) part 2: vT copies (ACT), final matmul, out
            if pending is not None:
                vT = []
                for k in range(10):
                    pst2, cols = pvT[k]
                    vk = vt.tile([P, P], hp, tag=f"vT{k}")
                    act.copy(vk[:cols, :], pst2[:cols, :])
                    vT.append((vk, cols))
                pout = ps_out.tile([P, D_MODEL], f32, tag="pout")
                for k in range(10):
                    vk, rows = vT[k]
                    lwk, rows2 = lw[k]
                    nc.tensor.matmul(pout[:], vk[:rows, :], lwk[:rows, :],
                                     start=(k == 0), stop=(k == 9))
                out_sb = outp.tile([P, D_MODEL], f32, tag="out_sb")
                act.copy(out_sb[:], pout[:])
                nc.sync.dma_start(out_dram[pit * P:(pit + 1) * P, :], out_sb[:])

            # Pool backend (P side)
            backend(pool, pbigP, eP0v, eP1v, HD, H, t6P, t2P, a0P, a0Pb)
            a0Pv = a0P[:].rearrange("p (h d q) -> p h d q", h=HP, d=D)
            attP = att_all[:, HD:].rearrange("p h (d q) -> p h d q", d=D)
            r_bP = rPv.unsqueeze(2).broadcast_to([P, HP, D, QJ])
            corr_att(pool, attP, a0Pv, r_bP, duP, va_v[:, HD:, :, 0], ctP, HP)

            # --- DVE D-side rest ---
            emult(eaD, ebD, eD0, eD1, HD)
            rowsum(eD0, eD1, r12D, r6D, r2D, sD, HD)
            with nc.allow_low_precision(reason="fp16 attention"):
                dve.reciprocal(r16D[:], sD[:])
            eD0v = eD0[:].rearrange("p (h q k) -> p h q k", h=HD, q=QJ)
            eD1v = eD1[:].rearrange("p (h q k) -> p h q k", h=HD, q=QJ)
            rDv = r16D[:].rearrange("p (h q) -> p h q", h=HD)
            dve.tensor_tensor(uD[:].rearrange("p (h q) -> p h q", h=HD),
                              eD0v[:, :, :, 0], rDv, AL.mult)
            act.copy(u2D[:], uD[:])
            chains(dve, e_resolver(eD0v, eD1v), HD, 0, tmp8D)
            chains(dve, flat_resolver(u2D[:].rearrange("p (h q) -> p h q", h=HD)),
                   HD, 0, tmp8D)
            dve.tensor_tensor(duD[:], u2D[:], uD[:], AL.subtract)
            backend(dve, pbigD, eD0v, eD1v, 0, HD, s2aD, r12D, a0D, a0Db)
            a0Dv = a0D[:].rearrange("p (h d q) -> p h d q", h=HD, d=D)
            attD = att_all[:, :HD].rearrange("p h (d q) -> p h d q", d=D)
            r_bD = rDv.unsqueeze(2).broadcast_to([P, HD, D, QJ])
            corr_att(dve, attD, a0Dv, r_bD, duD, va_v[:, :HD, :, 0], ctD, HD)

            pending = (vptok, it)

        # epilogue flush for the last tile
        pvpt, pit = pending
        vT = []
        for k in range(10):
            cols = min(P, DH + 1 - k * P)
            pst2 = ps_t.tile([P, P], hp, tag="pst2")
            nc.tensor.transpose(pst2[:cols, :], pvpt[:, k * P:k * P + cols], ident[:])
            vk = vt.tile([P, P], hp, tag=f"vT{k}")
            act.copy(vk[:cols, :], pst2[:cols, :])
            vT.append((vk, cols))
        pout = ps_out.tile([P, D_MODEL], f32, tag="pout")
        for k in range(10):
            vk, rows = vT[k]
            lwk, rows2 = lw[k]
            nc.tensor.matmul(pout[:], vk[:rows, :], lwk[:rows, :],
                             start=(k == 0), stop=(k == 9))
        out_sb = outp.tile([P, D_MODEL], f32, tag="out_sb")
        act.copy(out_sb[:], pout[:])
        nc.sync.dma_start(out_dram[pit * P:(pit + 1) * P, :], out_sb[:])

    nc.compile()
    return nc


def prep_weights(qk_w, v_w, lin_w, lin_b):
    scale = np.float32(1.0 / np.sqrt(6.0))
    wq = np.asarray(qk_w[:, :DH], dtype=np.float32).reshape(D_MODEL, H, 147)
    wk = np.asarray(qk_w[:, DH:], dtype=np.float32).reshape(D_MODEL, H, 147)
    wv = np.asarray(v_w, dtype=np.float32).reshape(D_MODEL, H, 147)
    wq_p = (wq[:, :, 3:] * scale).reshape(D_MODEL, H * NQK)
    wk_p = wk[:, :, 3:].reshape(D_MODEL, H * NQK)
    wv_att = wv[:, :, 3:].reshape(D_MODEL, H, KJ, D).transpose(0, 1, 3, 2).reshape(D_MODEL, H * NQK)
    wv_pass = wv[:, :, :3].reshape(D_MODEL, H * 3)
    w_cat = np.ascontiguousarray(
        np.concatenate([wq_p, wk_p, wv_att, wv_pass], axis=1)).astype(np.float16)
    # lin_w rows permuted to the (h, [pass3, d*24+q]) vptok layout + bias row
    lwr = np.asarray(lin_w, dtype=np.float32).reshape(H, 147, D_MODEL)
    att = lwr[:, 3:, :].reshape(H, QJ, D, D_MODEL).transpose(0, 2, 1, 3).reshape(H, NQK, D_MODEL)
    lw_p = np.concatenate([lwr[:, :3, :], att], axis=1).reshape(DH, D_MODEL)
    lw_aug = np.zeros((1184, D_MODEL), dtype=np.float32)
    lw_aug[:DH] = lw_p
    lw_aug[DH] = np.asarray(lin_b, dtype=np.float32)
    return w_cat, np.ascontiguousarray(lw_aug).astype(np.float16)


def make_in_maps(query, value, qk_w, v_w, lin_w, lin_b):
    w_cat, lw_aug = prep_weights(qk_w, v_w, lin_w, lin_b)
    q = np.asarray(query, dtype=np.float32)
    v = np.asarray(value, dtype=np.float32)
    bpc = B // N_CORES
    in_maps = []
    for c in range(N_CORES):
        qc = q[c * bpc:(c + 1) * bpc].reshape(-1, D_MODEL).T
        vc = v[c * bpc:(c + 1) * bpc].reshape(-1, D_MODEL).T
        in_maps.append({
            "qT": np.ascontiguousarray(qc).astype(np.float16),
            "vT": np.ascontiguousarray(vc).astype(np.float16),
            "w_cat": w_cat,
            "lin_w": lw_aug,
        })
    return in_maps


_CACHED_NC = None


def _get_nc():
    global _CACHED_NC
    if _CACHED_NC is None:
        _CACHED_NC = build_program(TT)
    return _CACHED_NC


def kernel(query, key, value, qk_w, v_w, lin_w, lin_b, _want_results=False, **_ignored):
    """Full-input kernel: shards batch over 8 cores, returns full output."""
    in_maps = make_in_maps(query, value, qk_w, v_w, lin_w, lin_b)
    nc = _get_nc()
    bpc = B // N_CORES
    res = run_bass_kernel_spmd(nc, in_maps, core_ids=list(range(N_CORES)))
    out = np.empty((B, N, D_MODEL), dtype=np.float32)
    for c in range(N_CORES):
        out[c * bpc:(c + 1) * bpc] = res.results[c]["out"].reshape(bpc, N, D_MODEL)
    if _want_results:
        return out, res
    return out


# revision 13
# speedup vs baseline: 1.2640x; 1.0003x over previous
"""Trainium2 Bass kernel for nn_DS_Attention_7636451852327.

Data-parallel over batch: 32 batches -> 8 NeuronCores, 4 batches (2048 tokens)
per core, 16 token-tiles of 128.

Host-side prep: q/v shipped pre-transposed ([512, T] fp16) so the QKV matmul
lhsT tiles are direct DMA loads (no on-device cast / PE transpose / PSUM
copy).  lin_w rows are permuted so the attention output is written in
(h, d, q) order (keeps the normalization multiply in DVE 2x mode), and the
output bias is folded into the final matmul via an appended ones-row.

Engine split (vertical, by head): DVE runs heads [0, HD) end-to-end plus the
front-end (products/d-sum/rowsum) of the Pool-side heads [HD, 8); the Pool
(GPSIMD) engine runs the back-end of those heads (PV products, k-sum tree,
normalize-by-divide, corrections). Pool only ever consumes DVE-produced
data -- DVE never waits on Pool mid-tile -- and every tile crossing the
engine boundary is double-buffered, so the two engines pipeline cleanly
about half a tile apart.  ACT does PSUM evictions and the exp()s.
"""
import os as _os
import numpy as np
from contextlib import ExitStack

import concourse.bass as bass
import concourse.mybir as mybir
import concourse.tile as tile
from concourse import bacc
from concourse.bass_utils import run_bass_kernel_spmd
from concourse.masks import make_identity

hp = mybir.dt.float16
f32 = mybir.dt.float32
AL = mybir.AluOpType
AX = mybir.AxisListType
AF = mybir.ActivationFunctionType

P = 128
H = 8
QJ = KJ = 24
D = 6
NQK = QJ * D              # 144
NVA = H * NQK             # 1152
NP = QJ * KJ              # 576 (q,k) pairs per head
DH = 1176                 # 147*8
D_MODEL = 512
W_TOT = 3 * NVA + H * 3   # 3480
B = 32
N = 512
N_CORES = 8
TT = (B // N_CORES) * N // P   # 16 token tiles per core

# custom-weighting chain levels: dst col range <- src col range (per head)
CH_LEVELS = (((6, 7), (3, 4)), ((9, 10), (6, 7)),
             ((12, 15), (9, 10)), ((15, 18), (12, 15)))

QKV_CHUNKS = [
    (0, 512, 0), (512, 1024, 0), (1024, 1152, 0),
    (1152, 1664, 1), (1664, 2176, 1), (2176, 2304, 1),
    (2304, 2816, 2), (2816, 3328, 2), (3328, 3480, 2),
]


def _cfg(name, default):
    v = _os.environ.get(name)
    return int(v) if v else default

HD = _cfg("HD", 5)        # DVE-side head count; Pool back-end owns the rest
HP = H - HD


def build_program(tt=TT, inner_repeat=1):
    nc = bacc.Bacc("TRN2", target_bir_lowering=False, debug=False)
    T = tt * P
    qT_dram = nc.dram_tensor("qT", [D_MODEL, T], hp, kind="ExternalInput").ap()
    vT_dram = nc.dram_tensor("vT", [D_MODEL, T], hp, kind="ExternalInput").ap()
    wcat_dram = nc.dram_tensor("w_cat", [D_MODEL, W_TOT], hp, kind="ExternalInput").ap()
    lw_dram = nc.dram_tensor("lin_w", [1184, D_MODEL], hp, kind="ExternalInput").ap()
    out_dram = nc.dram_tensor("out", [T, D_MODEL], f32, kind="ExternalOutput").ap()

    dve, pool, act = nc.vector, nc.gpsimd, nc.scalar

    with tile.TileContext(nc) as tc, ExitStack() as ctx:
        const = ctx.enter_context(tc.tile_pool(name="const", bufs=1))
        wpool = ctx.enter_context(tc.tile_pool(name="wpool", bufs=1))
        io = ctx.enter_context(tc.tile_pool(name="io", bufs=2))
        qkv = ctx.enter_context(tc.tile_pool(name="qkv", bufs=1))
        vab = ctx.enter_context(tc.tile_pool(name="vab", bufs=2))
        vpt = ctx.enter_context(tc.tile_pool(name="vpt", bufs=2))
        bigD = ctx.enter_context(tc.tile_pool(name="bigD", bufs=1))
        bigP = ctx.enter_context(tc.tile_pool(name="bigP", bufs=2))
        poolP = ctx.enter_context(tc.tile_pool(name="poolP", bufs=1))
        small = ctx.enter_context(tc.tile_pool(name="small", bufs=1))
        smx = ctx.enter_context(tc.tile_pool(name="smx", bufs=2))
        vt = ctx.enter_context(tc.tile_pool(name="vt", bufs=1))
        outp = ctx.enter_context(tc.tile_pool(name="outp", bufs=2))
        ps_t = ctx.enter_context(tc.tile_pool(name="ps_t", bufs=2, space="PSUM"))
        ps_mm = ctx.enter_context(tc.tile_pool(name="ps_mm", bufs=2, space="PSUM"))
        ps_out = ctx.enter_context(tc.tile_pool(name="ps_out", bufs=2, space="PSUM"))

        ident = const.tile([P, P], hp, tag="ident")
        make_identity(nc, ident[:])
        wcat = []
        for k in range(4):
            wk = wpool.tile([P, W_TOT], hp, tag=f"wcat{k}")
            nc.sync.dma_start(wk[:], wcat_dram[k * P:(k + 1) * P, :])
            wcat.append(wk)
        lw = []
        for k in range(10):
            rows = min(P, DH - k * P)
            if k == 9:
                rows += 1  # bias row
            lwk = wpool.tile([P, D_MODEL], hp, tag=f"lw{k}")
            nc.sync.dma_start(lwk[:rows, :], lw_dram[k * P:k * P + rows, :])
            lw.append((lwk, rows))

        pending = None
        for it in range(tt):
          for _rep in range(inner_repeat):
            # ---- input tiles: direct transposed fp16 loads ----
            xq, xv = [], []
            for src, dst, nm in ((qT_dram, xq, "q"), (vT_dram, xv, "v")):
                for k in range(4):
                    xk = io.tile([P, P], hp, tag=f"x{nm}{k}")
                    nc.sync.dma_start(xk[:], src[k * P:(k + 1) * P, it * P:(it + 1) * P])
                    dst.append(xk)

            # ---- QKV projection: q/k chunks first (ACT copies feed DVE) ----
            qa_all = qkv.tile([P, NVA], hp, tag="qa_all")
            ka_all = qkv.tile([P, NVA], hp, tag="ka_all")
            va_all = vab.tile([P, NVA], hp, tag="va_all")
            vptok = vpt.tile([P, DH + 1], hp, tag="vptok")
            pool.memset(vptok[:, DH:DH + 1], 1.0)  # ones col -> bias row of v'^T

            def qkv_chunk(c0, c1, kind):
                w_n = c1 - c0
                pmm = ps_mm.tile([P, 512], f32, tag="pmm")
                lhs_tiles = xv if kind == 2 else xq
                for k in range(4):
                    nc.tensor.matmul(pmm[:, :w_n], lhs_tiles[k][:], wcat[k][:, c0:c1],
                                     start=(k == 0), stop=(k == 3))
                if kind == 0:
                    act.copy(qa_all[:, c0:c1], pmm[:, :w_n])
                elif kind == 1:
                    act.copy(ka_all[:, c0 - NVA:c1 - NVA], pmm[:, :w_n])
                else:
                    v0, v1 = c0 - 2 * NVA, c1 - 2 * NVA
                    if v1 <= NVA:
                        act.copy(va_all[:, v0:v1], pmm[:, :w_n])
                    else:
                        act.copy(va_all[:, v0:NVA], pmm[:, :NVA - v0])
                        vp = pmm[:, NVA - v0:w_n].rearrange("p (h c) -> p h c", h=H)
                        vp_dst = vptok[:, :DH].rearrange("p (h c) -> p h c", h=H)[:, :, :3]
                        act.copy(vp_dst, vp)

            for (c0, c1, kind) in QKV_CHUNKS[:6]:
                qkv_chunk(c0, c1, kind)

            qa_v = qa_all[:].rearrange("p (h q d) -> p h q d", h=H, q=QJ)
            ka_v = ka_all[:].rearrange("p (h k d) -> p h k d", h=H, k=KJ)
            va_v = va_all[:].rearrange("p (h d k) -> p h d k", h=H, d=D)
            att_all = vptok[:, :DH].rearrange("p (h c) -> p h c", h=H)[:, :, 3:]

            # epilogue(i-1) part 1: PE transposes of previous tile's v'
            if pending is not None:
                pvpt, pit = pending
                pvT = []
                for k in range(10):
                    cols = min(P, DH + 1 - k * P)
                    pst2 = ps_t.tile([P, P], hp, tag="pst2")
                    nc.tensor.transpose(pst2[:cols, :], pvpt[:, k * P:k * P + cols], ident[:])
                    pvT.append((pst2, cols))

            # ---- per-side tiles ----
            CD = min(HD, 3)                     # p1 group capacity (heads)
            CP = min(HP, 2)
            pbigD = bigD.tile([P, CD * NP * D], hp, tag="pbigD")
            pbigP = bigP.tile([P, CP * NP * D], hp, tag="pbigP")
            s2aD = bigD.tile([P, HD * NP * 2], hp, tag="s2aD")
            s2aP = bigD.tile([P, HP * NP * 2], hp, tag="s2aP")
            eaD = bigD.tile([P, HD * NP], hp, tag="eaD")
            ebD = bigD.tile([P, HD * NP], hp, tag="ebD")
            eD0 = bigD.tile([P, HD * QJ * 12], hp, tag="eD0")
            eD1 = bigD.tile([P, HD * QJ * 12], hp, tag="eD1")
            eaP = bigD.tile([P, HP * NP], hp, tag="eaP")
            ebP = bigD.tile([P, HP * NP], hp, tag="ebP")
            eP0 = bigP.tile([P, HP * QJ * 12], hp, tag="eP0")
            eP1 = bigP.tile([P, HP * QJ * 12], hp, tag="eP1")
            t6P = poolP.tile([P, HP * NQK * 6], hp, tag="t6P")
            t2P = poolP.tile([P, HP * NQK * 2], hp, tag="t2P")
            a0P = poolP.tile([P, HP * NQK], hp, tag="a0P")
            a0Pb = poolP.tile([P, HP * NQK], hp, tag="a0Pb")
            ctP = poolP.tile([P, HP * D * D], hp, tag="ctP")
            r12D = small.tile([P, HD * QJ * 12], hp, tag="r12D")
            r6D = small.tile([P, HD * QJ * 6], hp, tag="r6D")
            r2D = small.tile([P, HD * QJ * 2], hp, tag="r2D")
            sD = small.tile([P, HD * QJ], f32, tag="sD")
            r16D = small.tile([P, HD * QJ], hp, tag="r16D")
            r12P = small.tile([P, HP * QJ * 12], hp, tag="r12P")
            r6P = small.tile([P, HP * QJ * 6], hp, tag="r6P")
            r2P = small.tile([P, HP * QJ * 2], hp, tag="r2P")
            sP = small.tile([P, HP * QJ], f32, tag="sP")
            rP16 = smx.tile([P, HP * QJ], hp, tag="rP16")
            a0D = small.tile([P, HD * NQK], hp, tag="a0D")
            a0Db = small.tile([P, HD * NQK], hp, tag="a0Db")
            ctD = small.tile([P, HD * D * D], hp, tag="ctD")
            uD = small.tile([P, HD * QJ], hp, tag="uD")
            u2D = small.tile([P, HD * QJ], hp, tag="u2D")
            duD = small.tile([P, HD * QJ], hp, tag="duD")
            uP = small.tile([P, HP * QJ], hp, tag="uP")
            u2P = small.tile([P, HP * QJ], hp, tag="u2P")
            duP = smx.tile([P, HP * QJ], hp, tag="duP")
            tmp8D = small.tile([P, H * 3], hp, tag="tmp8D")
            tmp8P = poolP.tile([P, H * 3], hp, tag="tmp8P")

            s2Dv = s2aD[:].rearrange("p (h pr e) -> p h pr e", h=HD, pr=NP)
            s2Pv = s2aP[:].rearrange("p (h pr e) -> p h pr e", h=HP, pr=NP)

            def p1_s2a(pb, s2t, g0, g1, o):
                nh = g1 - g0
                p1v = pb[:, o * NP * D:(o + nh) * NP * D].rearrange(
                    "p (h q k d) -> p h q k d", h=nh, q=QJ, k=KJ)
                p1f = pb[:, o * NP * D:(o + nh) * NP * D].rearrange(
                    "p (h pr d) -> p h pr d", h=nh, pr=NP)
                qa_b = qa_v[:, g0:g1].unsqueeze(3).broadcast_to([P, nh, QJ, KJ, D])
                ka_b = ka_v[:, g0:g1].unsqueeze(2).broadcast_to([P, nh, QJ, KJ, D])
                dve.tensor_tensor(p1v, qa_b, ka_b, AL.mult)
                rel = g0 - (0 if s2t is s2aD else HD)
                s2v = s2t[:, rel * NP * 2:(rel + nh) * NP * 2].rearrange(
                    "p (h pr e) -> p h pr e", h=nh, pr=NP)
                dve.tensor_tensor(s2v, p1f[:, :, :, 0:2], p1f[:, :, :, 2:4], AL.add)
                dve.tensor_tensor(s2v, s2v, p1f[:, :, :, 4:6], AL.add)

            def emult(ea_t, eb_t, e0, e1, nh):
                eav = ea_t[:].rearrange("p (h q k) -> p h q k", h=nh, q=QJ)
                ebv = eb_t[:].rearrange("p (h q k) -> p h q k", h=nh, q=QJ)
                for kh, et in ((0, e0), (1, e1)):
                    dve.tensor_tensor(
                        et[:].rearrange("p (h q k) -> p h q k", h=nh, q=QJ),
                        eav[:, :, :, kh * 12:(kh + 1) * 12],
                        ebv[:, :, :, kh * 12:(kh + 1) * 12], AL.mult)

            def rowsum(e0, e1, r12t, r6t, r2t, st, nh):
                fq = nh * QJ
                r12v = r12t[:].rearrange("p (f k) -> p f k", f=fq)
                r6v = r6t[:].rearrange("p (f k) -> p f k", f=fq)
                r2v = r2t[:].rearrange("p (f k) -> p f k", f=fq)
                dve.tensor_tensor(r12t[:], e0[:], e1[:], AL.add)
                dve.tensor_tensor(r6v, r12v[:, :, 0:6], r12v[:, :, 6:12], AL.add)
                dve.tensor_tensor(r2v, r6v[:, :, 0:2], r6v[:, :, 2:4], AL.add)
                dve.tensor_tensor(r2v, r2v, r6v[:, :, 4:6], AL.add)
                with nc.allow_low_precision(reason="fp16 attention"):
                    dve.tensor_tensor(st[:], r2v[:, :, 0], r2v[:, :, 1], AL.add)

            def chains(eng, resolve, nh, toff, tmp):
                t3 = tmp[:, toff * 3:(toff + nh) * 3].rearrange("p (h c) -> p h c", h=nh)
                for dsl, ssl in CH_LEVELS:
                    nd = dsl[1] - dsl[0]
                    dst = resolve(dsl[0], dsl[1])
                    srcv = resolve(ssl[0], ssl[1])
                    if ssl[1] - ssl[0] < nd:
                        srcv = srcv.broadcast_to([P, nh, nd])
                    eng.tensor_tensor(t3[:, :, :nd], dst, srcv, AL.add)
                    eng.tensor_scalar_mul(dst, t3[:, :, :nd], 0.5)

            def e_resolver(e0v, e1v):
                def resolve(c0, c1):
                    if c1 <= 12:
                        return e0v[:, :, 0, c0:c1]
                    return e1v[:, :, 0, c0 - 12:c1 - 12]
                return resolve

            def flat_resolver(v):
                return lambda c0, c1: v[:, :, c0:c1]

            def backend(eng, pb, e0v, e1v, g0, g1, t6t, t2t, a0t, a0bt):
                nh = g1 - g0
                for kh, a0o in ((0, a0t), (1, a0bt)):
                    ehv = e0v if kh == 0 else e1v
                    p2v = pb[:, :nh * NQK * 12].rearrange(
                        "p (h d q k) -> p h d q k", h=nh, d=D, q=QJ)
                    if eng is pool:
                        for hh in range(nh):
                            e_b = ehv[:, hh].unsqueeze(1).broadcast_to([P, D, QJ, 12])
                            va_b = va_v[:, g0 + hh, :, kh * 12:(kh + 1) * 12
                                        ].unsqueeze(2).broadcast_to([P, D, QJ, 12])
                            eng.tensor_tensor(p2v[:, hh], e_b, va_b, AL.mult)
                    else:
                        e_b = ehv.unsqueeze(2).broadcast_to([P, nh, D, QJ, 12])
                        va_b = va_v[:, g0:g1, :, kh * 12:(kh + 1) * 12].unsqueeze(
                            3).broadcast_to([P, nh, D, QJ, 12])
                        eng.tensor_tensor(p2v, e_b, va_b, AL.mult)
                    p2f = pb[:, :nh * NQK * 12].rearrange(
                        "p (h f k) -> p h f k", h=nh, f=NQK)
                    t6v = t6t[:, :nh * NQK * 6].rearrange("p (h f k) -> p h f k", h=nh, f=NQK)
                    t2v = t2t[:, :nh * NQK * 2].rearrange("p (h f k) -> p h f k", h=nh, f=NQK)
                    eng.tensor_tensor(t6v, p2f[:, :, :, 0:6], p2f[:, :, :, 6:12], AL.add)
                    eng.tensor_tensor(t2v, t6v[:, :, :, 0:2], t6v[:, :, :, 2:4], AL.add)
                    eng.tensor_tensor(t2v, t2v, t6v[:, :, :, 4:6], AL.add)
                    eng.tensor_tensor(a0o[:].rearrange("p (h f) -> p h f", h=nh),
                                      t2v[:, :, :, 0], t2v[:, :, :, 1], AL.add)
                eng.tensor_tensor(a0t[:], a0t[:], a0bt[:], AL.add)

            def corr_att(eng, attv, a0v, r_b, du_t, va0, ct_t, nh):
                eng.tensor_tensor(attv, a0v, r_b, AL.mult)
                du_v = du_t[:].rearrange("p (h q) -> p h q", h=nh)
                ctv = ct_t[:].rearrange("p (h d q) -> p h d q", h=nh, d=D)
                for qsl, n_q in (((12, 18, 1), 6), ((6, 10, 3), 2)):
                    du_b = du_v[:, :, qsl[0]:qsl[1]:qsl[2]].unsqueeze(2).broadcast_to(
                        [P, nh, D, n_q])
                    va0_b = va0.unsqueeze(3).broadcast_to([P, nh, D, n_q])
                    eng.tensor_tensor(ctv[:, :, :, :n_q], du_b, va0_b, AL.mult)
                    eng.tensor_tensor(attv[:, :, :, qsl[0]:qsl[1]:qsl[2]],
                                      attv[:, :, :, qsl[0]:qsl[1]:qsl[2]],
                                      ctv[:, :, :, :n_q], AL.add)

            # --- DVE front-end: P side first (advances the Pool start gate) ---
            p1_s2a(pbigP, s2aP, HD, HD + CP, 0)
            act.activation(eaP[:, :CP * NP], s2Pv[:, :CP, :, 0], AF.Exp)
            act.activation(ebP[:, :CP * NP], s2Pv[:, :CP, :, 1], AF.Exp)
            if HP > CP:
                p1_s2a(pbigP, s2aP, HD + CP, H, 0)
                act.activation(eaP[:, CP * NP:], s2Pv[:, CP:, :, 0], AF.Exp)
                act.activation(ebP[:, CP * NP:], s2Pv[:, CP:, :, 1], AF.Exp)
            p1_s2a(pbigD, s2aD, 0, CD, 0)
            act.activation(eaD[:, :CD * NP], s2Dv[:, :CD, :, 0], AF.Exp)
            act.activation(ebD[:, :CD * NP], s2Dv[:, :CD, :, 1], AF.Exp)

            emult(eaP, ebP, eP0, eP1, HP)
            rowsum(eP0, eP1, r12P, r6P, r2P, sP, HP)
            with nc.allow_low_precision(reason="fp16 attention"):
                dve.reciprocal(rP16[:], sP[:])
            eP0v = eP0[:].rearrange("p (h q k) -> p h q k", h=HP, q=QJ)
            eP1v = eP1[:].rearrange("p (h q k) -> p h q k", h=HP, q=QJ)
            rPv = rP16[:].rearrange("p (h q) -> p h q", h=HP)
            dve.tensor_tensor(uP[:].rearrange("p (h q) -> p h q", h=HP),
                              eP0v[:, :, :, 0], rPv, AL.mult)
            dve.tensor_copy(u2P[:], uP[:])
            chains(dve, flat_resolver(u2P[:].rearrange("p (h q) -> p h q", h=HP)),
                   HP, HD, tmp8D)
            dve.tensor_tensor(duP[:], u2P[:], uP[:], AL.subtract)
            # Pool may start its side now (last tile: keep on DVE, no tail)
            peng = dve if it == tt - 1 else pool
            chains(peng, e_resolver(eP0v, eP1v), HP, 0, tmp8P)

            # --- DVE D-side front-end ---
            if HD > CD:
                p1_s2a(pbigD, s2aD, CD, HD, 0)
                act.activation(eaD[:, CD * NP:], s2Dv[:, CD:, :, 0], AF.Exp)
                act.activation(ebD[:, CD * NP:], s2Dv[:, CD:, :, 1], AF.Exp)

            # va chunks (ACT after the exps) -- Pool p2 needs them
            for (c0, c1, kind) in QKV_CHUNKS[6:]:
                qkv_chunk(c0, c1, kind)

            # epilogue(i-1) part 2: vT copies (ACT), final matmul, out
            if pending is not None:
                vT = []
                for k in range(10):
                    pst2, cols = pvT[k]
                    vk = vt.tile([P, P], hp, tag=f"vT{k}")
                    act.copy(vk[:cols, :], pst2[:cols, :])
                    vT.append((vk, cols))
                pout = ps_out.tile([P, D_MODEL], f32, tag="pout")
                for k in range(10):
                    vk, rows = vT[k]
                    lwk, rows2 = lw[k]
                    nc.tensor.matmul(pout[:], vk[:rows, :], lwk[:rows, :],
                                     start=(k == 0), stop=(k == 9))
                out_sb = outp.tile([P, D_MODEL], f32, tag="out_sb")
                act.copy(out_sb[:], pout[:])
                nc.sync.dma_start(out_dram[pit * P:(pit + 1) * P, :], out_sb[:])

            # Pool backend (P side)
            backend(peng, pbigP, eP0v, eP1v, HD, H, t6P, t2P, a0P, a0Pb)
            a0Pv = a0P[:].rearrange("p (h d q) -> p h d q", h=HP, d=D)
            attP = att_all[:, HD:].rearrange("p h (d q) -> p h d q", d=D)
            r_bP = rPv.unsqueeze(2).broadcast_to([P, HP, D, QJ])
            corr_att(peng, attP, a0Pv, r_bP, duP, va_v[:, HD:, :, 0], ctP, HP)

            # --- DVE D-side rest ---
            emult(eaD, ebD, eD0, eD1, HD)
            rowsum(eD0, eD1, r12D, r6D, r2D, sD, HD)
            with nc.allow_low_precision(reason="fp16 attention"):
                dve.reciprocal(r16D[:], sD[:])
            eD0v = eD0[:].rearrange("p (h q k) -> p h q k", h=HD, q=QJ)
            eD1v = eD1[:].rearrange("p (h q k) -> p h q k", h=HD, q=QJ)
            rDv = r16D[:].rearrange("p (h q) -> p h q", h=HD)
            dve.tensor_tensor(uD[:].rearrange("p (h q) -> p h q", h=HD),
                              eD0v[:, :, :, 0], rDv, AL.mult)
            dve.tensor_copy(u2D[:], uD[:])
            chains(dve, e_resolver(eD0v, eD1v), HD, 0, tmp8D)
            chains(dve, flat_resolver(u2D[:].rearrange("p (h q) -> p h q", h=HD)),
                   HD, 0, tmp8D)
            dve.tensor_tensor(duD[:], u2D[:], uD[:], AL.subtract)
            backend(dve, pbigD, eD0v, eD1v, 0, HD, s2aD, r12D, a0D, a0Db)
            a0Dv = a0D[:].rearrange("p (h d q) -> p h d q", h=HD, d=D)
            attD = att_all[:, :HD].rearrange("p h (d q) -> p h d q", d=D)
            r_bD = rDv.unsqueeze(2).broadcast_to([P, HD, D, QJ])
            corr_att(dve, attD, a0Dv, r_bD, duD, va_v[:, :HD, :, 0], ctD, HD)

            pending = (vptok, it)

        # epilogue flush for the last tile
        pvpt, pit = pending
        vT = []
        for k in range(10):
            cols = min(P, DH + 1 - k * P)
            pst2 = ps_t.tile([P, P], hp, tag="pst2")
            nc.tensor.transpose(pst2[:cols, :], pvpt[:, k * P:k * P + cols], ident[:])
            vk = vt.tile([P, P], hp, tag=f"vT{k}")
            act.copy(vk[:cols, :], pst2[:cols, :])
            vT.append((vk, cols))
        pout = ps_out.tile([P, D_MODEL], f32, tag="pout")
        for k in range(10):
            vk, rows = vT[k]
            lwk, rows2 = lw[k]
            nc.tensor.matmul(pout[:], vk[:rows, :], lwk[:rows, :],
                             start=(k == 0), stop=(k == 9))
        out_sb = outp.tile([P, D_MODEL], f32, tag="out_sb")
        act.copy(out_sb[:], pout[:])
        nc.sync.dma_start(out_dram[pit * P:(pit + 1) * P, :], out_sb[:])

    nc.compile()
    return nc


def prep_weights(qk_w, v_w, lin_w, lin_b):
    scale = np.float32(1.0 / np.sqrt(6.0))
    wq = np.asarray(qk_w[:, :DH], dtype=np.float32).reshape(D_MODEL, H, 147)
    wk = np.asarray(qk_w[:, DH:], dtype=np.float32).reshape(D_MODEL, H, 147)
    wv = np.asarray(v_w, dtype=np.float32).reshape(D_MODEL, H, 147)
    wq_p = (wq[:, :, 3:] * scale).reshape(D_MODEL, H * NQK)
    wk_p = wk[:, :, 3:].reshape(D_MODEL, H * NQK)
    wv_att = wv[:, :, 3:].reshape(D_MODEL, H, KJ, D).transpose(0, 1, 3, 2).reshape(D_MODEL, H * NQK)
    wv_pass = wv[:, :, :3].reshape(D_MODEL, H * 3)
    w_cat = np.ascontiguousarray(
        np.concatenate([wq_p, wk_p, wv_att, wv_pass], axis=1)).astype(np.float16)
    # lin_w rows permuted to the (h, [pass3, d*24+q]) vptok layout + bias row
    lwr = np.asarray(lin_w, dtype=np.float32).reshape(H, 147, D_MODEL)
    att = lwr[:, 3:, :].reshape(H, QJ, D, D_MODEL).transpose(0, 2, 1, 3).reshape(H, NQK, D_MODEL)
    lw_p = np.concatenate([lwr[:, :3, :], att], axis=1).reshape(DH, D_MODEL)
    lw_aug = np.zeros((1184, D_MODEL), dtype=np.float32)
    lw_aug[:DH] = lw_p
    lw_aug[DH] = np.asarray(lin_b, dtype=np.float32)
    return w_cat, np.ascontiguousarray(lw_aug).astype(np.float16)


def make_in_maps(query, value, qk_w, v_w, lin_w, lin_b):
    w_cat, lw_aug = prep_weights(qk_w, v_w, lin_w, lin_b)
    q = np.asarray(query, dtype=np.float32)
    v = np.asarray(value, dtype=np.float32)
    bpc = B // N_CORES
    in_maps = []
    for c in range(N_CORES):
        qc = q[c * bpc:(c + 1) * bpc].reshape(-1, D_MODEL).T
        vc = v[c * bpc:(c + 1) * bpc].reshape(-1, D_MODEL).T
        in_maps.append({
            "qT": np.ascontiguousarray(qc).astype(np.float16),
            "vT": np.ascontiguousarray(vc).astype(np.float16),
            "w_cat": w_cat,
            "lin_w": lw_aug,
        })
    return in_maps


_CACHED_NC = None


def _get_nc():
    global _CACHED_NC
    if _CACHED_NC is None:
        _CACHED_NC = build_program(TT)
    return _CACHED_NC


def kernel(query, key, value, qk_w, v_w, lin_w, lin_b, _want_results=False, **_ignored):
    """Full-input kernel: shards batch over 8 cores, returns full output."""
    in_maps = make_in_maps(query, value, qk_w, v_w, lin_w, lin_b)
    nc = _get_nc()
    bpc = B // N_CORES
    res = run_bass_kernel_spmd(nc, in_maps, core_ids=list(range(N_CORES)))
    out = np.empty((B, N, D_MODEL), dtype=np.float32)
    for c in range(N_CORES):
        out[c * bpc:(c + 1) * bpc] = res.results[c]["out"].reshape(bpc, N, D_MODEL)
    if _want_results:
        return out, res
    return out
